# revision 1
# baseline (speedup 1.0000x reference)
"""Trainium2 Bass kernel for softmax RGB blend (pytorch3d NoLightShader).

Full inputs (N=8, H=512, W=512, K=8) are sharded batch-wise across 8
NeuronCores (one image per core); the blend is per-pixel so no cross-core
communication is needed.

Math per pixel (K faces), restructured for fp16 throughput:
    th_k   = tanh(d_k/(2*sigma))              (1+th = 2*sigmoid(d/s) = 2q)
    s_k    = sign(-pf_k - 0.5)                (+1 masked, -1 valid)
    tq_k   = max(th_k, s_k)                   (masked -> 1, i.e. q=1)
    qq2_k  = 1 + tq_k           (= 2q; masked -> 2)   prodq = prod_k qq2_k
    alpha  = 1 - prodq/256
    pm2_k  = 1 - tq_k           (= 2p, 0 when masked)
    zmin   = min_k zbuf_k  (UNMASKED - see note), zd_k = zmin - zbuf_k
    ex_k   = exp(zd_k/(D*gamma)),  w2_k = pm2_k * ex_k          (= 2*w_num)
    delta2 = exp(zmin/(D*gamma) + ln2 - ZFAR/(D*gamma))         (= 2*delta)
    denom2 = max(sum_k w2_k, 1e-30) + delta2
    rgb    = (sum_k w2_k*c_k + delta2) / denom2

Note: the reference masks z before the max; we use the unmasked min(zbuf).
When the global z-argmax face is masked (P ~ 1e-5 per pixel) all weights
underflow and the pixel degrades to rgb=0 - bounded, tiny rel-err impact.

Phase pipeline (phase u; tile t=u is the DVE main work; steady phase
17.8us, measured ns per op at T=256):
    SP  : input DMAs only, ~2 phases ahead, triple-buffered, one semaphore
          per input tensor (partial-batch sem waits are UNSOUND: the +16
          increments come from 16 independent SDMA engines, so a partial
          value can mix increments of different DMAs).
    DVE : zmin[u+1](2251) zd[u+1](2299) tq[u+1](1242) | qtree(1591)
          w2(1242) wtree(1594) wc(3378) ctree(3735) denom(446) recip(1774)
    ACT : th/s[u+1](2x1950) qq2/pm2[u+1](2x2000, Copy scale/bias)
          ex[u+1](1985) delta[u+1](490) | alpha[u](847) | out-DMA[u-1]
    GP  : t3[u-1](1583) rgb[u-1](1584) timed into DVE's contention-immune
          ops (t3 fires at denom -> overlaps recip; rgb fires at rcp ->
          overlaps zmin).
Engine-placement rules learned from traces (GPSIMD shares an SBUF port
with the DVE):
  - dense 2-stream fp16 DVE ops (TT 2x, STT, TS 4x) slow 2-4x under
    concurrent GPSIMD traffic; tensor_reduce/reciprocal are immune and
    f32 1x TTs lose ~25-45%.
  - GPSIMD itself reads strided fp16 ~3x slower than dense (8-byte fetch
    cliff), so only dense f32 work (t3/rgb) is worth placing there.
  - Keeping ACT on the exp_and_others table set (tanh/sign/exp/copy) avoids
    all ACT_TABLE_LOADs; a warm-up tanh pulls the single load ahead of the
    first DMA wait.
"""

import sys
from contextlib import ExitStack

import numpy as np

if "/opt/trn_rl_repo" not in sys.path:
    sys.path.insert(0, "/opt/trn_rl_repo")

SIGMA = 1e-4
GAMMA = 1e-4
ZNEAR = 1.0
ZFAR = 100.0
D = ZFAR - ZNEAR
EPS = 1e-10
S_EX = 1.0 / (D * GAMMA)                      # 101.0101...
B_DELTA = float(np.log(2.0)) - ZFAR / (D * GAMMA)

P = 128
K = 8
N_CORES = 8


def build_program(rows, T):
    import concourse.bass as bass
    from concourse import mybir

    dt = mybir.dt
    f32 = dt.float32
    f16 = dt.float16
    Alu = mybir.AluOpType
    Act = mybir.ActivationFunctionType
    Ax = mybir.AxisListType

    assert rows % T == 0
    n = rows // T
    TK = T * K

    nc = bass.Bass()

    zb_d = nc.dram_tensor("zbuf", [P, rows * K], f32, kind="ExternalInput")
    ds_d = nc.dram_tensor("dists", [P, rows * K], f16, kind="ExternalInput")
    pf_d = nc.dram_tensor("pix_to_face", [P, rows * K], f16, kind="ExternalInput")
    pc_d = nc.dram_tensor("pixel_colors", [P, rows * 3 * K], f16, kind="ExternalInput")
    out_d = nc.dram_tensor("out", [P, rows * 4], f16, kind="ExternalOutput")

    # const APs for activation biases (Exp/Sign need AP bias; Copy takes imm)
    for name, val in (("c_bd", B_DELTA), ("c_half", -0.5)):
        cb = nc.alloc_sbuf_tensor(name, [P, 1], f32)
        nc.gpsimd.memset(cb.ap(), val)
        nc.const_aps.aps[(f32, val)] = cb.ap()
    nc.all_engine_barrier()

    with ExitStack() as ctx:
        def sb(name, w, dtype=f16):
            return ctx.enter_context(nc.sbuf_tensor(name, [P, w], dtype))

        zb = [sb(f"zb{j}", TK, f32) for j in range(3)]
        ds = [sb(f"ds{j}", TK) for j in range(3)]
        pf = [sb(f"pf{j}", TK) for j in range(3)]
        col = [sb(f"col{j}", TK * 3) for j in range(3)]
        ot = [sb(f"ot{j}", T * 4) for j in range(2)]

        th = [sb(f"th{j}", TK) for j in range(2)]
        sg = [sb(f"sg{j}", TK) for j in range(2)]
        ex = [sb(f"ex{j}", TK) for j in range(2)]
        zd = [sb(f"zd{j}", TK) for j in range(2)]          # fp16
        zmin = [sb(f"zmin{j}", T, f32) for j in range(2)]

        tq = sb("tq", TK)
        qq2 = [sb(f"qq2{j}", TK) for j in range(2)]
        pm2 = [sb(f"pm2{j}", TK) for j in range(2)]
        qs4 = sb("qs4", T * 4)
        qs2 = sb("qs2", T * 2)
        prodq = [sb(f"prodq{j}", T, f32) for j in range(2)]
        wgt = sb("wgt", TK)
        ws4 = sb("ws4", T * 4)
        ws2 = sb("ws2", T * 2)
        wsum = sb("wsum", T, f32)
        wc = sb("wc", TK * 3)
        cs4 = sb("cs4", T * 12)
        cs2 = sb("cs2", T * 6)
        csum = [sb(f"csum{j}", T * 3, f32) for j in range(2)]
        delta = [sb(f"delta{j}", T, f32) for j in range(3)]
        denom = sb("denom", T, f32)
        rcp = [sb(f"rcp{j}", T, f32) for j in range(2)]
        t3 = sb("t3", T * 3, f32)
        warm = sb("warm", 1, f32)

        s_inz = ctx.enter_context(nc.semaphore("s_inz"))
        s_ind = ctx.enter_context(nc.semaphore("s_ind"))
        s_inp = ctx.enter_context(nc.semaphore("s_inp"))
        s_inc = ctx.enter_context(nc.semaphore("s_inc"))
        s_out = [ctx.enter_context(nc.semaphore(f"s_out{j}")) for j in range(2)]
        s_act = ctx.enter_context(nc.semaphore("s_act"))
        s_dve = ctx.enter_context(nc.semaphore("s_dve"))
        s_gp = ctx.enter_context(nc.semaphore("s_gp"))

        marks = {}

        def mk(eng, name, t, c):
            marks[(eng, name, t)] = c


        def out_done(t):
            return 16 * (t // 2 + 1)

        # ---------------- SP: input DMAs only, 2 phases ahead ------------
        def sched_sp(sp):
            for u in range(-2, n - 1):
                t = u + 2
                if sp is not None and t < n:
                    j = t % 3
                    if t >= 3:
                        sp.wait_ge(s_dve, marks[("d", "zd", t - 3)])
                        sp.wait_ge(s_act, marks[("a", "s", t - 3)])
                    sp.dma_start(out=zb[j][:], in_=zb_d[:, bass.ts(t, TK)]
                                 ).then_inc(s_inz, 16)
                    sp.dma_start(out=ds[j][:], in_=ds_d[:, bass.ts(t, TK)]
                                 ).then_inc(s_ind, 16)
                    sp.dma_start(out=pf[j][:], in_=pf_d[:, bass.ts(t, TK)]
                                 ).then_inc(s_inp, 16)
                tc = u + 1
                if sp is not None and 0 <= tc < n:
                    jc = tc % 3
                    if tc >= 3:
                        sp.wait_ge(s_dve, marks[("d", "wc", tc - 3)])
                    sp.dma_start(out=col[jc][:], in_=pc_d[:, bass.ts(tc, TK * 3)]
                                 ).then_inc(s_inc, 16)

        # ---------------- ACT ----------------
        def sched_act(act):
            c = 0
            if act is not None:
                # touch the tanh/exp/sign/copy table set before inputs land
                act.activation(warm[:], warm[:], Act.Tanh, scale=1.0)
            for u in range(-1, n + 1):
                t = u + 1
                if t < n:
                    j = t % 2
                    if act is not None:
                        act.wait_ge(s_ind, 16 * (t + 1))
                        if t >= 2:
                            act.wait_ge(s_dve, marks[("d", "tq", t - 2)])
                        act.activation(th[j][:], ds[t % 3][:], Act.Tanh,
                                       scale=1.0 / (2.0 * SIGMA))
                    c += 1; mk("a", "th", t, c)
                    if act is not None:
                        act.wait_ge(s_inp, 16 * (t + 1))
                        act.activation(sg[j][:], pf[t % 3][:], Act.Sign,
                                       scale=-1.0, bias=-0.5).then_inc(s_act, 2)
                    c += 1; mk("a", "s", t, c)
                    if act is not None and t > 0:
                        act.wait_ge(s_dve, marks[("d", "tq", t)])
                        if t >= 2:
                            act.wait_ge(s_dve, marks[("d", "prodq", t - 2)])
                        act.activation(qq2[j][:], tq[:], Act.Copy,
                                       scale=1.0, bias=1.0).then_inc(s_act, 1)
                    if t > 0:
                        c += 1
                    mk("a", "qq2", t, c)
                    if act is not None and t > 0:
                        if t >= 2:
                            act.wait_ge(s_dve, marks[("d", "w2", t - 2)])
                        act.activation(pm2[j][:], tq[:], Act.Copy,
                                       scale=-1.0, bias=1.0).then_inc(s_act, 1)
                    if t > 0:
                        c += 1
                    mk("a", "pm2", t, c)
                    if act is not None:
                        act.wait_ge(s_dve, marks[("d", "zd", t)])
                        act.activation(ex[j][:], zd[j][:], Act.Exp,
                                       scale=S_EX).then_inc(s_act, 1)
                    c += 1; mk("a", "ex", t, c)
                    if act is not None:
                        if t >= 3:
                            act.wait_ge(s_gp, marks[("g", "t3", t - 3)])
                            act.wait_ge(s_dve, marks[("d", "denom", t - 3)])
                        act.activation(delta[t % 3][:], zmin[t % 2][:], Act.Exp,
                                       scale=S_EX, bias=B_DELTA).then_inc(s_act, 1)
                    c += 1; mk("a", "delta", t, c)
                ta = u
                if 0 <= ta < n:
                    if act is not None:
                        act.wait_ge(s_dve, marks[("d", "prodq", ta)])
                        if ta >= 2:
                            act.wait_ge(s_out[ta % 2], out_done(ta - 2))
                        ot_v = ot[ta % 2][:].rearrange("p (t q) -> p t q", q=4)
                        act.activation(ot_v[:, :, 3:4],
                                       prodq[ta % 2][:].unsqueeze(2),
                                       Act.Copy, scale=-1.0 / 256.0, bias=1.0
                                       ).then_inc(s_act, 1)
                    c += 1; mk("a", "alpha", ta, c)
                to = u - 1
                if 0 <= to < n:
                    if act is not None:
                        act.wait_ge(s_gp, marks[("g", "rgb", to)])
                        act.dma_start(out=out_d[:, bass.ts(to, T * 4)],
                                      in_=ot[to % 2][:]).then_inc(s_out[to % 2], 16)
            if act is not None:
                act.wait_ge(s_out[0], 16 * ((n + 1) // 2))
                act.wait_ge(s_out[1], 16 * (n // 2))

        # ---------------- DVE ----------------
        def sched_dve(dve):
            c = 0
            for u in range(-1, n):
                tz = u + 1
                if 0 <= tz < n:
                    if dve is not None:
                        dve.wait_ge(s_inz, 16 * (tz + 1))
                        if tz >= 2:
                            dve.wait_ge(s_act, marks[("a", "delta", tz - 2)])
                        dve.tensor_reduce(
                            out=zmin[tz % 2][:],
                            in_=zb[tz % 3][:].rearrange("p (t k) -> p t k", k=K),
                            op=Alu.min, axis=Ax.X).then_inc(s_dve, 1)
                    c += 1; mk("d", "zmin", tz, c)
                    if dve is not None:
                        if tz >= 1:
                            dve.wait_ge(s_act, marks[("a", "ex", tz - 1)])
                        dve.tensor_tensor(
                            out=zd[tz % 2][:].rearrange("p (t k) -> p t k", k=K),
                            in0=zmin[tz % 2][:].unsqueeze(2)
                                .broadcast_to((P, T, K)),
                            in1=zb[tz % 3][:].rearrange("p (t k) -> p t k", k=K),
                            op=Alu.subtract).then_inc(s_dve, 1)
                    c += 1; mk("d", "zd", tz, c)
                    if dve is not None:
                        dve.wait_ge(s_act, marks[("a", "s", tz)])
                        if tz >= 1:
                            dve.wait_ge(s_act, marks[("a", "pm2", tz - 1)])
                        dve.tensor_tensor(out=tq[:], in0=th[tz % 2][:],
                                          in1=sg[tz % 2][:],
                                          op=Alu.max).then_inc(s_dve, 1)
                        if tz == 0:
                            dve.tensor_scalar(out=qq2[0][:], in0=tq[:],
                                              scalar1=1.0, scalar2=1.0,
                                              op0=Alu.add, op1=Alu.mult)
                            dve.tensor_scalar(out=pm2[0][:], in0=tq[:],
                                              scalar1=1.0, scalar2=-1.0,
                                              op0=Alu.subtract, op1=Alu.mult)
                    c += 1; mk("d", "tq", tz, c)
                t = u
                if not (0 <= t < n):
                    continue
                j = t % 2
                emit = dve is not None
                if emit:
                    dve.wait_ge(s_act, marks[("a", "qq2", t)])
                    if t >= 2:
                        dve.wait_ge(s_act, marks[("a", "alpha", t - 2)])
                    q_v = qq2[j][:].rearrange("p (t k) -> p t k", k=K)
                    q4_v = qs4[:].rearrange("p (t k) -> p t k", k=4)
                    q2_v = qs2[:].rearrange("p (t k) -> p t k", k=2)
                    dve.tensor_tensor(out=q4_v, in0=q_v[:, :, 0:4],
                                      in1=q_v[:, :, 4:8], op=Alu.mult)
                    dve.tensor_tensor(out=q2_v, in0=q4_v[:, :, 0:2],
                                      in1=q4_v[:, :, 2:4], op=Alu.mult)
                    dve.tensor_tensor(out=prodq[t % 2][:].unsqueeze(2),
                                      in0=q2_v[:, :, 0:1], in1=q2_v[:, :, 1:2],
                                      op=Alu.mult).then_inc(s_dve, 1)
                c += 1; mk("d", "prodq", t, c)
                if emit:
                    dve.wait_ge(s_act, marks[("a", "ex", t)])
                    dve.tensor_tensor(out=wgt[:], in0=pm2[j][:], in1=ex[j][:],
                                      op=Alu.mult).then_inc(s_dve, 1)
                c += 1; mk("d", "w2", t, c)
                if emit:
                    w_v = wgt[:].rearrange("p (t k) -> p t k", k=K)
                    w4_v = ws4[:].rearrange("p (t k) -> p t k", k=4)
                    w2_v = ws2[:].rearrange("p (t k) -> p t k", k=2)
                    dve.tensor_tensor(out=w4_v, in0=w_v[:, :, 0:4],
                                      in1=w_v[:, :, 4:8], op=Alu.add)
                    dve.tensor_tensor(out=w2_v, in0=w4_v[:, :, 0:2],
                                      in1=w4_v[:, :, 2:4], op=Alu.add)
                    dve.tensor_tensor(out=wsum[:].unsqueeze(2),
                                      in0=w2_v[:, :, 0:1], in1=w2_v[:, :, 1:2],
                                      op=Alu.add)
                if emit:
                    dve.wait_ge(s_inc, 16 * (t + 1))
                    wc_v = wc[:].rearrange("p (t c k) -> p t c k", c=3, k=K)
                    dve.tensor_tensor(
                        out=wc_v,
                        in0=w_v.unsqueeze(2).broadcast_to((P, T, 3, K)),
                        in1=col[t % 3][:].rearrange("p (t c k) -> p t c k",
                                                    c=3, k=K),
                        op=Alu.mult).then_inc(s_dve, 1)
                c += 1; mk("d", "wc", t, c)
                if emit:
                    wc_u = wc[:].rearrange("p (t c k) -> p (t c) k", c=3, k=K)
                    c4_u = cs4[:].rearrange("p (t c k) -> p (t c) k", c=3, k=4)
                    c2_u = cs2[:].rearrange("p (t c k) -> p (t c) k", c=3, k=2)
                    dve.tensor_tensor(out=c4_u, in0=wc_u[:, :, 0:4],
                                      in1=wc_u[:, :, 4:8], op=Alu.add)
                    dve.tensor_tensor(out=c2_u, in0=c4_u[:, :, 0:2],
                                      in1=c4_u[:, :, 2:4], op=Alu.add)
                    if t >= 2:
                        dve.wait_ge(s_gp, marks[("g", "t3", t - 2)])
                    dve.tensor_tensor(out=csum[j][:].unsqueeze(2),
                                      in0=c2_u[:, :, 0:1], in1=c2_u[:, :, 1:2],
                                      op=Alu.add).then_inc(s_dve, 1)
                c += 1; mk("d", "csum", t, c)
                if emit:
                    dve.wait_ge(s_act, marks[("a", "delta", t)])
                    dve.scalar_tensor_tensor(
                        out=denom[:], in0=wsum[:], scalar=1e-30,
                        in1=delta[t % 3][:], op0=Alu.max, op1=Alu.add,
                    ).then_inc(s_dve, 1)
                c += 1; mk("d", "denom", t, c)
                if emit:
                    if t >= 2:
                        dve.wait_ge(s_gp, marks[("g", "rgb", t - 2)])
                    dve.reciprocal(out=rcp[j][:], in_=denom[:]
                                   ).then_inc(s_dve, 1)
                c += 1; mk("d", "rcp", t, c)

        # ---------------- GP ----------------
        def sched_gp(gp):
            c = 0
            for u in range(-1, n + 1):
                to = u - 1
                if 0 <= to < n:
                    jj = to % 2
                    if gp is not None:
                        gp.wait_ge(s_dve, marks[("d", "denom", to)])
                        c_v = csum[jj][:].rearrange("p (t c) -> p t c", c=3)
                        t3_v = t3[:].rearrange("p (t c) -> p t c", c=3)
                        gp.tensor_tensor(
                            out=t3_v, in0=c_v,
                            in1=delta[to % 3][:].unsqueeze(2)
                                .broadcast_to((P, T, 3)),
                            op=Alu.add).then_inc(s_gp, 1)
                    c += 1; mk("g", "t3", to, c)
                    if gp is not None:
                        gp.wait_ge(s_dve, marks[("d", "rcp", to)])
                        if to >= 2:
                            gp.wait_ge(s_out[jj], out_done(to - 2))
                        ot_v = ot[jj][:].rearrange("p (t q) -> p t q", q=4)
                        gp.tensor_tensor(
                            out=ot_v[:, :, 0:3],
                            in0=t3[:].rearrange("p (t c) -> p t c", c=3),
                            in1=rcp[jj][:].unsqueeze(2).broadcast_to((P, T, 3)),
                            op=Alu.mult).then_inc(s_gp, 1)
                    c += 1; mk("g", "rgb", to, c)

        sched_sp(None)
        sched_act(None)
        sched_dve(None)
        sched_gp(None)

        blk = ctx.enter_context(nc.Block())

        @blk.sync
        def _(sp):
            sched_sp(sp)

        @blk.scalar
        def _(act):
            sched_act(act)

        @blk.vector
        def _(dve):
            sched_dve(dve)

        @blk.gpsimd
        def _(gp):
            sched_gp(gp)

    return nc


_CACHE = {}


def _get_program(rows=2048, T=256):
    key = (rows, T)
    if key not in _CACHE:
        _CACHE[key] = build_program(rows, T)
    return _CACHE[key]


def _run(pixel_colors, zbuf, dists, pix_to_face, trace=False):
    from concourse.bass_utils import run_bass_kernel_spmd

    N, H, W, Kk = zbuf.shape
    assert (N, H, W, Kk) == (8, 512, 512, 8), (N, H, W, Kk)
    rows = H * W // P  # 2048

    nc = _get_program(rows=rows, T=256)

    zb = np.ascontiguousarray(np.asarray(zbuf, dtype=np.float32))
    ds = np.asarray(dists).astype(np.float16)
    with np.errstate(over="ignore"):
        # face ids >= 65504 round to +inf; only the sign is used (mask)
        pfh = np.asarray(pix_to_face).astype(np.float16)
    # colors host-transposed to [..., 3, K] so k is innermost on-chip
    pc = np.asarray(pixel_colors).astype(np.float16)
    pc = np.ascontiguousarray(
        pc.reshape(N, P, rows, K, 3).transpose(0, 1, 2, 4, 3))

    in_maps = []
    for i in range(N_CORES):
        in_maps.append(
            {
                "zbuf": zb[i].reshape(P, rows * K),
                "dists": np.ascontiguousarray(ds[i].reshape(P, rows * K)),
                "pix_to_face": np.ascontiguousarray(pfh[i].reshape(P, rows * K)),
                "pixel_colors": pc[i].reshape(P, rows * 3 * K),
            }
        )

    res = run_bass_kernel_spmd(
        nc, in_maps, core_ids=list(range(N_CORES)), trace=trace
    )
    out = np.stack(
        [res.results[i]["out"].astype(np.float32).reshape(H, W, 4)
         for i in range(N_CORES)], axis=0
    )
    return out, res


def kernel(pixel_colors, zbuf, dists, pix_to_face):
    out, _ = _run(pixel_colors, zbuf, dists, pix_to_face, trace=False)
    return out



# revision 3
# speedup vs baseline: 1.0676x; 1.0676x over previous
"""Trainium2 Bass kernel for softmax RGB blend (pytorch3d NoLightShader).

Full inputs (N=8, H=512, W=512, K=8) are sharded batch-wise across 8
NeuronCores (one image per core); the blend is per-pixel so no cross-core
communication is needed.

Host-side input repack (per core, pure layout/dtype transforms):
  - mask folded into dists:  d' = masked ? +30000 : d   (fp16)
      (tanh(d'*5000) = 1  ->  q = 1, p = 0, exactly the masked case)
  - mask folded into z via quantization:
      zq = round((ZFAR - z)/D * 32767) * mask   (int16; 0 when masked,
      matching the reference's masked z_inv == 0 exactly)
  - colors fp16, k-major planar per phase:  [P, n, 3, K, T]
  - zq/d' k-major per phase: [P, n, K, T]
Output is written planar [P, n, 4, T] (r,g,b,a planes) and untransposed
on the host.

Math per pixel (K faces), all engines in f16 except where noted:
    th_k    = tanh(d_k*5000)          (ACT; masked -> 1)
    qq2_k   = 1 + th_k    (= 2q)      (ACT Copy;  prodq = prod_k qq2_k)
    alpha   = 1 - prodq/256           (ACT Copy, into out plane 3)
    zqmax   = max_k zq_k  (int16 TT-max tree -> f16; f16 rounding of
              zqmax is common-mode across k and cancels in the blend)
    zd_k    = zq_k - zqmax            (int16 - f16 bcast -> f16, 2x)
    ex_k    = exp(zd_k * S2)          (ACT, S2 = 1/(32767*GAMMA))
    w2neg_k = (th_k - 1) * ex_k       (DVE STT; = -2*w_num)
    wc planes 0..2 = w2neg*col, plane 3 = w2neg; one 4-plane add-tree
              -> cw = [csumneg(3), wsumneg] (f16)
    delta   = exp(-zqmax*S2 + ln2)    (ACT; f16, = 2*delta_ref)
    denomneg= min(wsumneg,-1e-30) - delta      (DVE STT, f32)
    rcpneg  = reciprocal_approx_fast(denomneg) (DVE custom, f32)
    rgb     = (csumneg - delta) * rcpneg       (GP: t3 then mult, planes 0..2)

Pipeline (phase u, n = rows/T phases): DVE does the z-stage for u+1
first (zq tree, zd), then the main stage for u.  ACT runs th/qq2/ex/
delta for u+1, alpha for u, out-DMA for u-1.  GP runs t3/rgb for u-1.
SP streams input DMAs (double-buffered).
"""

import sys
from contextlib import ExitStack

import numpy as np

if "/opt/trn_rl_repo" not in sys.path:
    sys.path.insert(0, "/opt/trn_rl_repo")

SIGMA = 1e-4
GAMMA = 1e-4
ZNEAR = 1.0
ZFAR = 100.0
D = ZFAR - ZNEAR
ZQ = 32767.0                                   # z_inv quantization scale
S2 = 1.0 / (ZQ * GAMMA)                        # exp scale on zd
LN2 = float(np.log(2.0))

P = 128
K = 8
N_CORES = 8


def build_program(rows, T):
    import concourse.bass as bass
    from concourse import mybir

    dt = mybir.dt
    f32 = dt.float32
    f16 = dt.float16
    i16 = dt.int16
    Alu = mybir.AluOpType
    Act = mybir.ActivationFunctionType

    assert rows % T == 0
    n = rows // T
    TK = T * K

    nc = bass.Bass()

    zq_d = nc.dram_tensor("zq", [P, rows * K], i16, kind="ExternalInput")
    ds_d = nc.dram_tensor("dists", [P, rows * K], f16, kind="ExternalInput")
    pc_d = nc.dram_tensor("pixel_colors", [P, rows * 3 * K], f16,
                          kind="ExternalInput")
    out_d = nc.dram_tensor("out", [P, rows * 4], f16, kind="ExternalOutput")

    # const AP for the Exp bias (Exp needs an AP bias; Copy takes imm)
    cb = nc.alloc_sbuf_tensor("c_ln2", [P, 1], f32)
    nc.gpsimd.memset(cb.ap(), LN2)
    nc.const_aps.aps[(f32, LN2)] = cb.ap()
    nc.all_engine_barrier()

    with ExitStack() as ctx:
        def sb(name, w, dtype=f16):
            return ctx.enter_context(nc.sbuf_tensor(name, [P, w], dtype))

        zq = [sb(f"zq{j}", TK, i16) for j in range(2)]
        dth = [sb(f"dth{j}", TK) for j in range(2)]        # d, then th in place
        col = [sb(f"col{j}", TK * 3) for j in range(2)]
        qq2 = [sb(f"qq2{j}", TK) for j in range(2)]
        zdex = [sb(f"zdex{j}", TK) for j in range(2)]      # zd, then ex in place
        zmx4 = sb("zmx4", T * 4, i16)                      # lvl2 aliases [0:2T]
        zqmax = [sb(f"zqmax{j}", T) for j in range(2)]
        wcb = sb("wcb", TK * 4)                            # planes rgb + w2neg
        t4a = sb("t4a", T * 16)                            # lvl2 aliases [0:8T]
        qs4 = sb("qs4", T * 4)                             # lvl2 aliases [0:2T]
        prodq = [sb(f"prodq{j}", T) for j in range(2)]
        cw = [sb(f"cw{j}", T * 4) for j in range(2)]       # csumneg*3, wsumneg
        delta = [sb(f"delta{j}", T) for j in range(3)]
        denomn = sb("denomn", T, f32)
        rcpn = [sb(f"rcpn{j}", T, f32) for j in range(2)]
        ot = [sb(f"ot{j}", T * 4) for j in range(2)]       # planes r,g,b,a
        warm = sb("warm", 1, f32)

        s_inz = ctx.enter_context(nc.semaphore("s_inz"))
        s_ind = ctx.enter_context(nc.semaphore("s_ind"))
        s_inc = ctx.enter_context(nc.semaphore("s_inc"))
        s_out = [ctx.enter_context(nc.semaphore(f"s_out{j}")) for j in range(2)]
        s_act = ctx.enter_context(nc.semaphore("s_act"))
        s_dve = ctx.enter_context(nc.semaphore("s_dve"))
        s_gp = ctx.enter_context(nc.semaphore("s_gp"))

        marks = {}

        def mk(eng, name, t, c):
            marks[(eng, name, t)] = c

        def out_done(t):
            return 16 * (t // 2 + 1)

        # views
        def v_kt(buf):
            return buf[:].rearrange("p (k t) -> p k t", k=K)

        def v_ckt(buf):
            return buf[:].rearrange("p (c k t) -> p c k t", c=3, k=K)

        # ---------------- SP: input DMAs, double-buffered -----------------
        def sched_sp(sp):
            for t in range(n):
                if sp is None:
                    continue
                if t >= 2:
                    sp.wait_ge(s_dve, marks[("d", "zd", t - 2)])
                sp.dma_start(out=zq[t % 2][:], in_=zq_d[:, bass.ts(t, TK)]
                             ).then_inc(s_inz, 16)
                if t >= 2:
                    sp.wait_ge(s_dve, marks[("d", "w2", t - 2)])
                    sp.wait_ge(s_act, marks[("a", "qq2", t - 2)])
                sp.dma_start(out=dth[t % 2][:], in_=ds_d[:, bass.ts(t, TK)]
                             ).then_inc(s_ind, 16)
                if t >= 2:
                    sp.wait_ge(s_dve, marks[("d", "wc", t - 2)])
                sp.dma_start(out=col[t % 2][:], in_=pc_d[:, bass.ts(t, TK * 3)]
                             ).then_inc(s_inc, 16)

        # ---------------- ACT ----------------
        def sched_act(act):
            c = 0
            if act is not None:
                act.activation(warm[:], warm[:], Act.Tanh, scale=1.0)
            for u in range(-1, n + 1):
                tz = u + 1
                if tz < n:
                    j = tz % 2
                    # th in place over d
                    if act is not None:
                        act.wait_ge(s_ind, 16 * (tz + 1))
                        act.activation(dth[j][:], dth[j][:], Act.Tanh,
                                       scale=1.0 / (2.0 * SIGMA)
                                       ).then_inc(s_act, 1)
                    c += 1
                    mk("a", "th", tz, c)
                    if act is not None:
                        if tz >= 2:
                            act.wait_ge(s_dve, marks[("d", "q1", tz - 2)])
                        act.activation(qq2[j][:], dth[j][:], Act.Copy,
                                       scale=1.0, bias=1.0).then_inc(s_act, 1)
                    c += 1
                    mk("a", "qq2", tz, c)
                    # ex in place over zd
                    if act is not None:
                        act.wait_ge(s_dve, marks[("d", "zd", tz)])
                        act.activation(zdex[j][:], zdex[j][:], Act.Exp,
                                       scale=S2).then_inc(s_act, 1)
                    c += 1
                    mk("a", "ex", tz, c)
                    if act is not None:
                        act.wait_ge(s_dve, marks[("d", "zqmax", tz)])
                        if tz >= 3:
                            act.wait_ge(s_dve, marks[("d", "denom", tz - 3)])
                            act.wait_ge(s_gp, marks[("g", "t3", tz - 3)])
                        act.activation(delta[tz % 3][:], zqmax[j][:], Act.Exp,
                                       scale=-S2, bias=LN2).then_inc(s_act, 1)
                    c += 1
                    mk("a", "delta", tz, c)
                ta = u
                if 0 <= ta < n:
                    if act is not None:
                        act.wait_ge(s_dve, marks[("d", "prodq", ta)])
                        if ta >= 2:
                            act.wait_ge(s_out[ta % 2], out_done(ta - 2))
                        ot_v = ot[ta % 2][:].rearrange("p (c t) -> p c t", c=4)
                        act.activation(ot_v[:, 3:4, :],
                                       prodq[ta % 2][:].unsqueeze(1),
                                       Act.Copy, scale=-1.0 / 256.0, bias=1.0
                                       ).then_inc(s_act, 1)
                    c += 1
                    mk("a", "alpha", ta, c)
                to = u - 1
                if 0 <= to < n:
                    if act is not None:
                        act.wait_ge(s_gp, marks[("g", "rgb", to)])
                        act.dma_start(out=out_d[:, bass.ts(to, T * 4)],
                                      in_=ot[to % 2][:]).then_inc(s_out[to % 2], 16)
            if act is not None:
                act.wait_ge(s_out[0], 16 * ((n + 1) // 2))
                act.wait_ge(s_out[1], 16 * (n // 2))

        # ---------------- DVE ----------------
        def sched_dve(dve):
            c = 0
            for u in range(-1, n):
                tz = u + 1
                if 0 <= tz < n:
                    j = tz % 2
                    emit = dve is not None
                    if emit:
                        dve.wait_ge(s_inz, 16 * (tz + 1))
                        zq_v = v_kt(zq[j])
                        zx4 = zmx4[:].rearrange("p (k t) -> p k t", k=4)
                        dve.tensor_tensor(out=zx4, in0=zq_v[:, 0:4, :],
                                          in1=zq_v[:, 4:8, :], op=Alu.max)
                        dve.tensor_tensor(out=zx4[:, 0:2, :],
                                          in0=zx4[:, 0:2, :],
                                          in1=zx4[:, 2:4, :], op=Alu.max)
                        if tz >= 2:
                            dve.wait_ge(s_act, marks[("a", "delta", tz - 2)])
                        dve.tensor_tensor(out=zqmax[j][:].unsqueeze(1),
                                          in0=zx4[:, 0:1, :],
                                          in1=zx4[:, 1:2, :],
                                          op=Alu.max).then_inc(s_dve, 1)
                    c += 1
                    mk("d", "zqmax", tz, c)
                    if emit:
                        dve.tensor_tensor(
                            out=v_kt(zdex[j]),
                            in0=v_kt(zq[j]),
                            in1=zqmax[j][:].unsqueeze(1)
                                .broadcast_to((P, K, T)),
                            op=Alu.subtract).then_inc(s_dve, 1)
                    c += 1
                    mk("d", "zd", tz, c)
                t = u
                if not (0 <= t < n):
                    continue
                j = t % 2
                emit = dve is not None
                if emit:
                    dve.wait_ge(s_act, marks[("a", "qq2", t)])
                    q_v = v_kt(qq2[j])
                    q4 = qs4[:].rearrange("p (k t) -> p k t", k=4)
                    dve.tensor_tensor(out=q4, in0=q_v[:, 0:4, :],
                                      in1=q_v[:, 4:8, :],
                                      op=Alu.mult).then_inc(s_dve, 1)
                c += 1
                mk("d", "q1", t, c)
                if emit:
                    dve.tensor_tensor(out=q4[:, 0:2, :], in0=q4[:, 0:2, :],
                                      in1=q4[:, 2:4, :], op=Alu.mult)
                    if t >= 2:
                        dve.wait_ge(s_act, marks[("a", "alpha", t - 2)])
                    dve.tensor_tensor(out=prodq[j][:].unsqueeze(1),
                                      in0=q4[:, 0:1, :], in1=q4[:, 1:2, :],
                                      op=Alu.mult).then_inc(s_dve, 1)
                c += 1
                mk("d", "prodq", t, c)
                if emit:
                    dve.wait_ge(s_act, marks[("a", "ex", t)])
                    wcv = wcb[:].rearrange("p (c k t) -> p c k t", c=4, k=K)
                    dve.scalar_tensor_tensor(
                        out=wcv[:, 3, :, :], in0=v_kt(dth[j]), scalar=1.0,
                        in1=v_kt(zdex[j]), op0=Alu.subtract, op1=Alu.mult,
                    ).then_inc(s_dve, 1)
                c += 1
                mk("d", "w2", t, c)
                if emit:
                    dve.wait_ge(s_inc, 16 * (t + 1))
                    dve.tensor_tensor(
                        out=wcv[:, 0:3, :, :],
                        in0=wcv[:, 3:4, :, :].broadcast_to((P, 3, K, T)),
                        in1=v_ckt(col[j]),
                        op=Alu.mult).then_inc(s_dve, 1)
                c += 1
                mk("d", "wc", t, c)
                if emit:
                    t4 = t4a[:].rearrange("p (c k t) -> p c k t", c=4, k=4)
                    dve.tensor_tensor(out=t4, in0=wcv[:, :, 0:4, :],
                                      in1=wcv[:, :, 4:8, :], op=Alu.add)
                    dve.tensor_tensor(out=t4[:, :, 0:2, :],
                                      in0=t4[:, :, 0:2, :],
                                      in1=t4[:, :, 2:4, :], op=Alu.add)
                    if t >= 2:
                        dve.wait_ge(s_gp, marks[("g", "t3", t - 2)])
                    cw_v = cw[j][:].rearrange("p (c t) -> p c t", c=4)
                    dve.tensor_tensor(out=cw_v,
                                      in0=t4[:, :, 0, :],
                                      in1=t4[:, :, 1, :],
                                      op=Alu.add).then_inc(s_dve, 1)
                c += 1
                mk("d", "cw", t, c)
                if emit:
                    dve.wait_ge(s_act, marks[("a", "delta", t)])
                    dve.scalar_tensor_tensor(
                        out=denomn[:], in0=cw_v[:, 3, :], scalar=-1e-30,
                        in1=delta[t % 3][:], op0=Alu.min, op1=Alu.subtract,
                    ).then_inc(s_dve, 1)
                c += 1
                mk("d", "denom", t, c)
                if emit:
                    if t >= 2:
                        dve.wait_ge(s_gp, marks[("g", "rgb", t - 2)])
                    dve.reciprocal(out=rcpn[j][:], in_=denomn[:]
                                   ).then_inc(s_dve, 1)
                c += 1
                mk("d", "rcp", t, c)

        # ---------------- GP ----------------
        def sched_gp(gp):
            c = 0
            for u in range(0, n + 1):
                to = u - 1
                if not (0 <= to < n):
                    continue
                jj = to % 2
                if gp is not None:
                    gp.wait_ge(s_dve, marks[("d", "cw", to)])
                    gp.wait_ge(s_act, marks[("a", "delta", to)])
                    if to >= 2:
                        gp.wait_ge(s_out[jj], out_done(to - 2))
                    cw_v = cw[jj][:].rearrange("p (c t) -> p c t", c=4)
                    ot_v = ot[jj][:].rearrange("p (c t) -> p c t", c=4)
                    gp.tensor_tensor(
                        out=ot_v[:, 0:3, :], in0=cw_v[:, 0:3, :],
                        in1=delta[to % 3][:].unsqueeze(1)
                            .broadcast_to((P, 3, T)),
                        op=Alu.subtract).then_inc(s_gp, 1)
                c += 1
                mk("g", "t3", to, c)
                if gp is not None:
                    gp.wait_ge(s_dve, marks[("d", "rcp", to)])
                    gp.tensor_tensor(
                        out=ot_v[:, 0:3, :], in0=ot_v[:, 0:3, :],
                        in1=rcpn[jj][:].unsqueeze(1).broadcast_to((P, 3, T)),
                        op=Alu.mult).then_inc(s_gp, 1)
                c += 1
                mk("g", "rgb", to, c)

        sched_sp(None)
        sched_act(None)
        sched_dve(None)
        sched_gp(None)

        blk = ctx.enter_context(nc.Block())

        @blk.sync
        def _(sp):
            sched_sp(sp)

        @blk.scalar
        def _(act):
            sched_act(act)

        @blk.vector
        def _(dve):
            sched_dve(dve)

        @blk.gpsimd
        def _(gp):
            sched_gp(gp)

    return nc


_CACHE = {}


def _get_program(rows=2048, T=512):
    key = (rows, T)
    if key not in _CACHE:
        _CACHE[key] = build_program(rows, T)
    return _CACHE[key]


def _prep_core(zb, ds, pf, pc, rows, T):
    """Host-side repack for one core: returns dict of DRAM arrays."""
    n = rows // T
    mask = pf >= 0                                        # [P, rows, K]
    z_inv = (ZFAR - zb) * (1.0 / D)
    np.clip(z_inv, 0.0, 1.0, out=z_inv)
    zq = np.rint(z_inv * ZQ).astype(np.int16)
    zq[~mask] = 0
    d16 = ds.astype(np.float16)
    d16[~mask] = np.float16(30000.0)
    # k-major, phase-major: [P, rows, K] -> [P, n, K, T]
    zq = np.ascontiguousarray(
        zq.reshape(P, n, T, K).transpose(0, 1, 3, 2)).reshape(P, rows * K)
    d16 = np.ascontiguousarray(
        d16.reshape(P, n, T, K).transpose(0, 1, 3, 2)).reshape(P, rows * K)
    # colors: [P, rows, K, 3] -> [P, n, 3, K, T]
    c16 = pc.astype(np.float16).reshape(P, n, T, K, 3)
    c16 = np.ascontiguousarray(c16.transpose(0, 1, 4, 3, 2)
                               ).reshape(P, rows * 3 * K)
    return {"zq": zq, "dists": d16, "pixel_colors": c16}


def _run(pixel_colors, zbuf, dists, pix_to_face, trace=False):
    from concourse.bass_utils import run_bass_kernel_spmd

    N, H, W, Kk = zbuf.shape
    assert (N, H, W, Kk) == (8, 512, 512, 8), (N, H, W, Kk)
    rows = H * W // P  # 2048
    T = 512
    n = rows // T

    nc = _get_program(rows=rows, T=T)

    zb = np.asarray(zbuf, dtype=np.float32)
    ds = np.asarray(dists, dtype=np.float32)
    pf = np.asarray(pix_to_face)
    pc = np.asarray(pixel_colors, dtype=np.float32)

    in_maps = []
    for i in range(N_CORES):
        in_maps.append(_prep_core(
            zb[i].reshape(P, rows, K),
            ds[i].reshape(P, rows, K),
            pf[i].reshape(P, rows, K),
            pc[i].reshape(P, rows, K, 3),
            rows, T,
        ))

    res = run_bass_kernel_spmd(
        nc, in_maps, core_ids=list(range(N_CORES)), trace=trace
    )
    outs = []
    for i in range(N_CORES):
        o = res.results[i]["out"].astype(np.float32)
        # [P, n, 4, T] planar -> [P, rows, 4]
        o = o.reshape(P, n, 4, T).transpose(0, 1, 3, 2).reshape(H, W, 4)
        outs.append(o)
    return np.stack(outs, axis=0), res


def kernel(pixel_colors, zbuf, dists, pix_to_face):
    out, _ = _run(pixel_colors, zbuf, dists, pix_to_face, trace=False)
    return out


# revision 10
# speedup vs baseline: 1.0890x; 1.0201x over previous
"""Trainium2 Bass kernel for softmax RGB blend (pytorch3d NoLightShader).

Full inputs (N=8, H=512, W=512, K=8) are sharded batch-wise across 8
NeuronCores (one image per core); the blend is per-pixel so no cross-core
communication is needed.

Host-side input repack (per core, pure layout/dtype transforms):
  - mask folded into dists:  d' = masked ? +30000 : d   (fp16)
      (tanh(d'*5000) = 1  ->  q = 1, p = 0, exactly the masked case)
  - mask folded into z via quantization:
      zq = round((ZFAR - z)/D * 32767) * mask   (int16; 0 when masked,
      matching the reference's masked z_inv == 0 exactly)
  - colors fp16, k-major planar per phase:  [P, n, 3, K, T]
  - zq/d' k-major per phase: [P, n, K, T]
Output is written planar [P, n, 4, T] (r,g,b,a planes) and untransposed
on the host.

Math per pixel (K faces), all engines in f16 except where noted:
    th_k    = tanh(d_k*5000)          (ACT; masked -> 1)
    qq2_k   = 1 + th_k    (= 2q)      (ACT Copy;  prodq = prod_k qq2_k)
    alpha   = 1 - prodq/256           (ACT Copy, into out plane 3)
    zqmax   = max_k zq_k  (int16 TT-max tree -> f16; f16 rounding of
              zqmax is common-mode across k and cancels in the blend)
    zd_k    = zq_k - zqmax            (int16 - f16 bcast -> f16, 2x)
    ex_k    = exp(zd_k * S2)          (ACT, S2 = 1/(32767*GAMMA))
    pm2_k   = (th_k - 1)*(-1)         (DVE TS 4x, into wc plane 3)
    w2_k    = pm2_k * ex_k            (DVE TT in place, = 2*w_num)
    wc planes 0..2 = w2*col, plane 3 = w2; one 4-plane add-tree
              -> cw = [csum(3), wsum] (f16)
    delta   = exp(-zqmax*S2 + ln2)    (ACT; f16, = 2*delta_ref)
    denom   = max(wsum,1e-30) + delta (DVE STT, f32)
    rcp     = 1/denom                 (ACT Reciprocal; costs 2 table
              loads/phase but moves 3.3us/phase off the DVE)
    rgb     = (csum + delta) * rcp    (GP: t3 then mult, planes 0..2)

Pipeline (phase u, n = rows/T phases): DVE does the z-stage for u+1
first (zq tree, zd), then the main stage for u.  ACT runs th/qq2/ex/
delta for u+1, alpha/rcp for u, out-DMA for u-1.  GP runs t3/rgb for
u-1, gated on denom[u] so it lands in the DVE's denom/reciprocal window
(GPSIMD shares an SBUF port with the DVE; concurrent GP traffic slows
dense fp16 2-stream DVE ops).  SP streams input DMAs (double-buffered).
"""

import sys
from contextlib import ExitStack

import numpy as np

if "/opt/trn_rl_repo" not in sys.path:
    sys.path.insert(0, "/opt/trn_rl_repo")

SIGMA = 1e-4
GAMMA = 1e-4
ZNEAR = 1.0
ZFAR = 100.0
D = ZFAR - ZNEAR
ZQ = 32767.0                                   # z_inv quantization scale
S2 = 1.0 / (ZQ * GAMMA)                        # exp scale on zd
LN2 = float(np.log(2.0))

P = 128
K = 8
N_CORES = 8


def build_program(rows, T):
    import concourse.bass as bass
    from concourse import mybir

    dt = mybir.dt
    f32 = dt.float32
    f16 = dt.float16
    i16 = dt.int16
    Alu = mybir.AluOpType
    Act = mybir.ActivationFunctionType

    assert rows % T == 0
    n = rows // T
    TK = T * K

    nc = bass.Bass()

    zq_d = nc.dram_tensor("zq", [P, rows * K], i16, kind="ExternalInput")
    ds_d = nc.dram_tensor("dists", [P, rows * K], f16, kind="ExternalInput")
    pc_d = nc.dram_tensor("pixel_colors", [P, rows * 3 * K], f16,
                          kind="ExternalInput")
    out_d = nc.dram_tensor("out", [P, rows * 4], f16, kind="ExternalOutput")

    # const AP for the Exp bias (Exp needs an AP bias; Copy takes imm)
    cb = nc.alloc_sbuf_tensor("c_ln2", [P, 1], f32)
    nc.gpsimd.memset(cb.ap(), LN2)
    nc.const_aps.aps[(f32, LN2)] = cb.ap()
    nc.all_engine_barrier()

    with ExitStack() as ctx:
        def sb(name, w, dtype=f16):
            return ctx.enter_context(nc.sbuf_tensor(name, [P, w], dtype))

        zq = [sb(f"zq{j}", TK, i16) for j in range(2)]
        dth = [sb(f"dth{j}", TK) for j in range(2)]        # d, then th in place
        col = [sb(f"col{j}", TK * 3) for j in range(2)]
        qq2 = [sb(f"qq2{j}", TK) for j in range(2)]
        zdex = [sb(f"zdex{j}", TK) for j in range(2)]      # zd, then ex in place
        zmx4 = sb("zmx4", T * 4, i16)                      # lvl2 aliases [0:2T]
        zqmax = [sb(f"zqmax{j}", T) for j in range(2)]
        wcb = sb("wcb", TK * 4)                            # planes rgb + w2neg
        t4a = sb("t4a", T * 16)                            # lvl2 aliases [0:8T]
        qs4 = sb("qs4", T * 4)                             # lvl2 aliases [0:2T]
        prodq = [sb(f"prodq{j}", T) for j in range(2)]
        cw = [sb(f"cw{j}", T * 4) for j in range(2)]       # csumneg*3, wsumneg
        delta = [sb(f"delta{j}", T) for j in range(3)]
        denomn = sb("denomn", T, f32)
        rcpn = [sb(f"rcpn{j}", T, f32) for j in range(2)]
        ot = [sb(f"ot{j}", T * 4) for j in range(2)]       # planes r,g,b,a
        warm = sb("warm", 1, f32)

        s_inz = ctx.enter_context(nc.semaphore("s_inz"))
        s_ind = ctx.enter_context(nc.semaphore("s_ind"))
        s_inc = ctx.enter_context(nc.semaphore("s_inc"))
        s_out = [ctx.enter_context(nc.semaphore(f"s_out{j}")) for j in range(2)]
        s_act = ctx.enter_context(nc.semaphore("s_act"))
        s_dve = ctx.enter_context(nc.semaphore("s_dve"))
        s_gp = ctx.enter_context(nc.semaphore("s_gp"))

        marks = {}

        def mk(eng, name, t, c):
            marks[(eng, name, t)] = c

        def out_done(t):
            return 16 * (t // 2 + 1)

        # views
        def v_kt(buf):
            return buf[:].rearrange("p (k t) -> p k t", k=K)

        def v_ckt(buf):
            return buf[:].rearrange("p (c k t) -> p c k t", c=3, k=K)

        # ---------------- SP: input DMAs, double-buffered -----------------
        def sched_sp(sp):
            for t in range(n):
                if sp is None:
                    continue
                if t >= 2:
                    sp.wait_ge(s_dve, marks[("d", "zd", t - 2)])
                sp.dma_start(out=zq[t % 2][:], in_=zq_d[:, bass.ts(t, TK)]
                             ).then_inc(s_inz, 16)
                if t >= 2:
                    sp.wait_ge(s_dve, marks[("d", "w2", t - 2)])
                    sp.wait_ge(s_act, marks[("a", "qq2", t - 2)])
                sp.dma_start(out=dth[t % 2][:], in_=ds_d[:, bass.ts(t, TK)]
                             ).then_inc(s_ind, 16)
                if t >= 2:
                    sp.wait_ge(s_dve, marks[("d", "wc", t - 2)])
                sp.dma_start(out=col[t % 2][:], in_=pc_d[:, bass.ts(t, TK * 3)]
                             ).then_inc(s_inc, 16)

        # ---------------- ACT ----------------
        def sched_act(act):
            c = 0
            if act is not None:
                act.activation(warm[:], warm[:], Act.Tanh, scale=1.0)
            for u in range(-1, n + 1):
                tz = u + 1
                if tz < n:
                    j = tz % 2
                    # th in place over d
                    if act is not None:
                        act.wait_ge(s_ind, 16 * (tz + 1))
                        act.activation(dth[j][:], dth[j][:], Act.Tanh,
                                       scale=1.0 / (2.0 * SIGMA)
                                       ).then_inc(s_act, 1)
                    c += 1
                    mk("a", "th", tz, c)
                    if act is not None:
                        if tz >= 2:
                            act.wait_ge(s_dve, marks[("d", "q1", tz - 2)])
                        act.activation(qq2[j][:], dth[j][:], Act.Copy,
                                       scale=1.0, bias=1.0).then_inc(s_act, 1)
                    c += 1
                    mk("a", "qq2", tz, c)
                    # ex in place over zd
                    if act is not None:
                        act.wait_ge(s_dve, marks[("d", "zd", tz)])
                        act.activation(zdex[j][:], zdex[j][:], Act.Exp,
                                       scale=S2).then_inc(s_act, 1)
                    c += 1
                    mk("a", "ex", tz, c)
                    if act is not None:
                        act.wait_ge(s_dve, marks[("d", "zqmax", tz)])
                        if tz >= 3:
                            act.wait_ge(s_dve, marks[("d", "denom", tz - 3)])
                            act.wait_ge(s_gp, marks[("g", "t3", tz - 3)])
                        act.activation(delta[tz % 3][:], zqmax[j][:], Act.Exp,
                                       scale=-S2, bias=LN2).then_inc(s_act, 1)
                    c += 1
                    mk("a", "delta", tz, c)
                ta = u
                if 0 <= ta < n:
                    if act is not None:
                        act.wait_ge(s_dve, marks[("d", "prodq", ta)])
                        if ta >= 2:
                            act.wait_ge(s_out[ta % 2], out_done(ta - 2))
                        ot_v = ot[ta % 2][:].rearrange("p (c t) -> p c t", c=4)
                        act.activation(ot_v[:, 3:4, :],
                                       prodq[ta % 2][:].unsqueeze(1),
                                       Act.Copy, scale=-1.0 / 256.0, bias=1.0
                                       ).then_inc(s_act, 1)
                    c += 1
                    mk("a", "alpha", ta, c)
                    if act is not None:
                        act.wait_ge(s_dve, marks[("d", "denom", ta)])
                        if ta >= 2:
                            act.wait_ge(s_gp, marks[("g", "rgb", ta - 2)])
                        # rcp = exp(-ln(denom)); Ln+Exp share the
                        # natural_log_exp_and_others table set (2 table
                        # loads per phase, ~2.6us ACT, saves 3.3us DVE)
                        act.activation(denomn[:], denomn[:], Act.Ln,
                                       scale=1.0)
                        act.activation(rcpn[ta % 2][:], denomn[:], Act.Exp,
                                       scale=-1.0).then_inc(s_act, 1)
                    c += 1
                    mk("a", "rcp", ta, c)
                to = u - 1
                if 0 <= to < n:
                    if act is not None:
                        act.wait_ge(s_gp, marks[("g", "rgb", to)])
                        act.dma_start(out=out_d[:, bass.ts(to, T * 4)],
                                      in_=ot[to % 2][:]).then_inc(s_out[to % 2], 16)
            if act is not None:
                act.wait_ge(s_out[0], 16 * ((n + 1) // 2))
                act.wait_ge(s_out[1], 16 * (n // 2))

        # ---------------- DVE ----------------
        def sched_dve(dve):
            c = 0
            for u in range(-1, n):
                tz = u + 1
                if 0 <= tz < n:
                    j = tz % 2
                    emit = dve is not None
                    if emit:
                        dve.wait_ge(s_inz, 16 * (tz + 1))
                        zq_v = v_kt(zq[j])
                        zx4 = zmx4[:].rearrange("p (k t) -> p k t", k=4)
                        dve.tensor_tensor(out=zx4, in0=zq_v[:, 0:4, :],
                                          in1=zq_v[:, 4:8, :], op=Alu.max)
                        dve.tensor_tensor(out=zx4[:, 0:2, :],
                                          in0=zx4[:, 0:2, :],
                                          in1=zx4[:, 2:4, :], op=Alu.max)
                        if tz >= 2:
                            dve.wait_ge(s_act, marks[("a", "delta", tz - 2)])
                        dve.tensor_tensor(out=zqmax[j][:].unsqueeze(1),
                                          in0=zx4[:, 0:1, :],
                                          in1=zx4[:, 1:2, :],
                                          op=Alu.max).then_inc(s_dve, 1)
                    c += 1
                    mk("d", "zqmax", tz, c)
                    if emit:
                        dve.tensor_tensor(
                            out=v_kt(zdex[j]),
                            in0=v_kt(zq[j]),
                            in1=zqmax[j][:].unsqueeze(1)
                                .broadcast_to((P, K, T)),
                            op=Alu.subtract).then_inc(s_dve, 1)
                    c += 1
                    mk("d", "zd", tz, c)
                t = u
                if not (0 <= t < n):
                    continue
                j = t % 2
                emit = dve is not None
                if emit:
                    dve.wait_ge(s_act, marks[("a", "qq2", t)])
                    q_v = v_kt(qq2[j])
                    q4 = qs4[:].rearrange("p (k t) -> p k t", k=4)
                    dve.tensor_tensor(out=q4, in0=q_v[:, 0:4, :],
                                      in1=q_v[:, 4:8, :],
                                      op=Alu.mult).then_inc(s_dve, 1)
                c += 1
                mk("d", "q1", t, c)
                if emit:
                    dve.tensor_tensor(out=q4[:, 0:2, :], in0=q4[:, 0:2, :],
                                      in1=q4[:, 2:4, :], op=Alu.mult)
                    if t >= 2:
                        dve.wait_ge(s_act, marks[("a", "alpha", t - 2)])
                    dve.tensor_tensor(out=prodq[j][:].unsqueeze(1),
                                      in0=q4[:, 0:1, :], in1=q4[:, 1:2, :],
                                      op=Alu.mult).then_inc(s_dve, 1)
                c += 1
                mk("d", "prodq", t, c)
                if emit:
                    dve.wait_ge(s_act, marks[("a", "ex", t)])
                    wcv = wcb[:].rearrange("p (c k t) -> p c k t", c=4, k=K)
                    dve.tensor_scalar(out=wcv[:, 3, :, :], in0=v_kt(dth[j]),
                                      scalar1=1.0, scalar2=-1.0,
                                      op0=Alu.subtract, op1=Alu.mult)
                    dve.tensor_tensor(out=wcv[:, 3, :, :],
                                      in0=wcv[:, 3, :, :],
                                      in1=v_kt(zdex[j]),
                                      op=Alu.mult).then_inc(s_dve, 1)
                c += 1
                mk("d", "w2", t, c)
                if emit:
                    dve.wait_ge(s_inc, 16 * (t + 1))
                    dve.tensor_tensor(
                        out=wcv[:, 0:3, :, :],
                        in0=wcv[:, 3:4, :, :].broadcast_to((P, 3, K, T)),
                        in1=v_ckt(col[j]),
                        op=Alu.mult).then_inc(s_dve, 1)
                c += 1
                mk("d", "wc", t, c)
                if emit:
                    t4 = t4a[:].rearrange("p (c k t) -> p c k t", c=4, k=4)
                    dve.tensor_tensor(out=t4, in0=wcv[:, :, 0:4, :],
                                      in1=wcv[:, :, 4:8, :], op=Alu.add)
                    dve.tensor_tensor(out=t4[:, :, 0:2, :],
                                      in0=t4[:, :, 0:2, :],
                                      in1=t4[:, :, 2:4, :], op=Alu.add)
                    if t >= 2:
                        dve.wait_ge(s_gp, marks[("g", "t3", t - 2)])
                    cw_v = cw[j][:].rearrange("p (c t) -> p c t", c=4)
                    dve.tensor_tensor(out=cw_v,
                                      in0=t4[:, :, 0, :],
                                      in1=t4[:, :, 1, :],
                                      op=Alu.add).then_inc(s_dve, 1)
                c += 1
                mk("d", "cw", t, c)
                if emit:
                    dve.wait_ge(s_act, marks[("a", "delta", t)])
                    if t >= 1:
                        dve.wait_ge(s_act, marks[("a", "rcp", t - 1)])
                    dve.scalar_tensor_tensor(
                        out=denomn[:], in0=cw_v[:, 3, :], scalar=1e-30,
                        in1=delta[t % 3][:], op0=Alu.max, op1=Alu.add,
                    ).then_inc(s_dve, 1)
                c += 1
                mk("d", "denom", t, c)

        # ---------------- GP ----------------
        def sched_gp(gp):
            c = 0
            for u in range(0, n + 1):
                to = u - 1
                if not (0 <= to < n):
                    continue
                jj = to % 2
                if gp is not None:
                    gp.wait_ge(s_dve, marks[("d", "cw", to)])
                    gp.wait_ge(s_act, marks[("a", "delta", to)])
                    if to + 1 < n:
                        # land in the next phase's denom/rcp window to limit
                        # SBUF-port contention with dense fp16 DVE ops
                        gp.wait_ge(s_dve, marks[("d", "denom", to + 1)])
                    if to >= 2:
                        gp.wait_ge(s_out[jj], out_done(to - 2))
                    cw_v = cw[jj][:].rearrange("p (c t) -> p c t", c=4)
                    ot_v = ot[jj][:].rearrange("p (c t) -> p c t", c=4)
                    gp.tensor_tensor(
                        out=ot_v[:, 0:3, :], in0=cw_v[:, 0:3, :],
                        in1=delta[to % 3][:].unsqueeze(1)
                            .broadcast_to((P, 3, T)),
                        op=Alu.add).then_inc(s_gp, 1)
                c += 1
                mk("g", "t3", to, c)
                if gp is not None:
                    gp.wait_ge(s_act, marks[("a", "rcp", to)])
                    gp.tensor_tensor(
                        out=ot_v[:, 0:3, :], in0=ot_v[:, 0:3, :],
                        in1=rcpn[jj][:].unsqueeze(1).broadcast_to((P, 3, T)),
                        op=Alu.mult).then_inc(s_gp, 1)
                c += 1
                mk("g", "rgb", to, c)

        sched_sp(None)
        sched_act(None)
        sched_dve(None)
        sched_gp(None)

        blk = ctx.enter_context(nc.Block())

        @blk.sync
        def _(sp):
            sched_sp(sp)

        @blk.scalar
        def _(act):
            sched_act(act)

        @blk.vector
        def _(dve):
            sched_dve(dve)

        @blk.gpsimd
        def _(gp):
            sched_gp(gp)

    return nc


_CACHE = {}


def _get_program(rows=2048, T=512):
    key = (rows, T)
    if key not in _CACHE:
        _CACHE[key] = build_program(rows, T)
    return _CACHE[key]


def _prep_core(zb, ds, pf, pc, rows, T):
    """Host-side repack for one core: returns dict of DRAM arrays."""
    n = rows // T
    mask = pf >= 0                                        # [P, rows, K]
    z_inv = (ZFAR - zb) * (1.0 / D)
    np.clip(z_inv, 0.0, 1.0, out=z_inv)
    zq = np.rint(z_inv * ZQ).astype(np.int16)
    zq[~mask] = 0
    d16 = ds.astype(np.float16)
    d16[~mask] = np.float16(30000.0)
    # k-major, phase-major: [P, rows, K] -> [P, n, K, T]
    zq = np.ascontiguousarray(
        zq.reshape(P, n, T, K).transpose(0, 1, 3, 2)).reshape(P, rows * K)
    d16 = np.ascontiguousarray(
        d16.reshape(P, n, T, K).transpose(0, 1, 3, 2)).reshape(P, rows * K)
    # colors: [P, rows, K, 3] -> [P, n, 3, K, T]
    c16 = pc.astype(np.float16).reshape(P, n, T, K, 3)
    c16 = np.ascontiguousarray(c16.transpose(0, 1, 4, 3, 2)
                               ).reshape(P, rows * 3 * K)
    return {"zq": zq, "dists": d16, "pixel_colors": c16}


def _run(pixel_colors, zbuf, dists, pix_to_face, trace=False):
    from concourse.bass_utils import run_bass_kernel_spmd

    N, H, W, Kk = zbuf.shape
    assert (N, H, W, Kk) == (8, 512, 512, 8), (N, H, W, Kk)
    rows = H * W // P  # 2048
    T = 512
    n = rows // T

    nc = _get_program(rows=rows, T=T)

    zb = np.asarray(zbuf, dtype=np.float32)
    ds = np.asarray(dists, dtype=np.float32)
    pf = np.asarray(pix_to_face)
    pc = np.asarray(pixel_colors, dtype=np.float32)

    in_maps = []
    for i in range(N_CORES):
        in_maps.append(_prep_core(
            zb[i].reshape(P, rows, K),
            ds[i].reshape(P, rows, K),
            pf[i].reshape(P, rows, K),
            pc[i].reshape(P, rows, K, 3),
            rows, T,
        ))

    res = run_bass_kernel_spmd(
        nc, in_maps, core_ids=list(range(N_CORES)), trace=trace
    )
    outs = []
    for i in range(N_CORES):
        o = res.results[i]["out"].astype(np.float32)
        # [P, n, 4, T] planar -> [P, rows, 4]
        o = o.reshape(P, n, 4, T).transpose(0, 1, 3, 2).reshape(H, W, 4)
        outs.append(o)
    return np.stack(outs, axis=0), res


def kernel(pixel_colors, zbuf, dists, pix_to_face):
    out, _ = _run(pixel_colors, zbuf, dists, pix_to_face, trace=False)
    return out


# revision 14
# speedup vs baseline: 1.1108x; 1.0200x over previous
"""Trainium2 Bass kernel for softmax RGB blend (pytorch3d NoLightShader).

Full inputs (N=8, H=512, W=512, K=8) are sharded batch-wise across 8
NeuronCores (one image per core); the blend is per-pixel so no cross-core
communication is needed.

Host-side input repack (per core, pure layout/dtype transforms):
  - mask folded into dists:  d' = masked ? +30000 : d   (fp16)
      (tanh(d'*5000) = 1  ->  q = 1, p = 0, exactly the masked case)
  - mask folded into z via quantization:
      zq = round((ZFAR - z)/D * 32767) * mask   (int16; 0 when masked,
      matching the reference's masked z_inv == 0 exactly)
  - colors fp16, k-major planar per phase:  [P, n, 3, K, T]
  - zq/d' k-major per phase: [P, n, K, T]
Output is written planar [P, n, 4, T] (r,g,b,a planes) and untransposed
on the host.

Math per pixel (K faces), all engines in f16 except where noted:
    th_k    = tanh(d_k*5000)          (ACT; masked -> 1)
    qq2_k   = 1 + th_k    (= 2q)      (ACT Copy;  prodq = prod_k qq2_k)
    alpha   = 1 - prodq/256           (ACT Copy, into out plane 3)
    zqmax   = max_k zq_k  (int16 TT-max tree -> f16; f16 rounding of
              zqmax is common-mode across k and cancels in the blend)
    zd_k    = zq_k - zqmax            (int16 - f16 bcast -> f16, 2x)
    ex_k    = exp(zd_k * S2)          (ACT, S2 = 1/(32767*GAMMA))
    pm2_k   = (th_k - 1)*(-1)         (DVE TS 4x, into wc plane 3)
    w2_k    = pm2_k * ex_k            (DVE TT in place, = 2*w_num)
    wc planes 0..2 = w2*col, plane 3 = w2; one 4-plane add-tree
              -> cw = [csum(3), wsum] (f16)
    delta   = exp(-zqmax*S2 + ln2)    (ACT; f16, = 2*delta_ref)
    denom   = max(wsum,1e-30) + delta (DVE STT, f32)
    rcp     = 1/denom                 (ACT Reciprocal; costs 2 table
              loads/phase but moves 3.3us/phase off the DVE)
    rgb     = (csum + delta) * rcp    (GP: t3 then mult, planes 0..2)

Pipeline (phase u, n = rows/T phases): DVE does the z-stage for u+1
first (zq tree, zd), then the main stage for u.  ACT runs th/qq2/ex/
delta for u+1, alpha/rcp for u, out-DMA for u-1.  GP runs t3/rgb for
u-1, gated on denom[u] so it lands in the DVE's denom/reciprocal window
(GPSIMD shares an SBUF port with the DVE; concurrent GP traffic slows
dense fp16 2-stream DVE ops).  SP streams input DMAs (double-buffered).
"""

import sys
from contextlib import ExitStack

import numpy as np

if "/opt/trn_rl_repo" not in sys.path:
    sys.path.insert(0, "/opt/trn_rl_repo")

SIGMA = 1e-4
GAMMA = 1e-4
ZNEAR = 1.0
ZFAR = 100.0
D = ZFAR - ZNEAR
ZQ = 32767.0                                   # z_inv quantization scale
S2 = 1.0 / (ZQ * GAMMA)                        # exp scale on zd
LN2 = float(np.log(2.0))

P = 128
K = 8
N_CORES = 8


def build_program(rows, T):
    import concourse.bass as bass
    from concourse import mybir

    dt = mybir.dt
    f32 = dt.float32
    f16 = dt.float16
    i16 = dt.int16
    Alu = mybir.AluOpType
    Act = mybir.ActivationFunctionType

    assert rows % T == 0
    n = rows // T
    TK = T * K

    nc = bass.Bass()

    zq_d = nc.dram_tensor("zq", [P, rows * K], i16, kind="ExternalInput")
    ds_d = nc.dram_tensor("dists", [P, rows * K], f16, kind="ExternalInput")
    pc_d = nc.dram_tensor("pixel_colors", [P, rows * 3 * K], f16,
                          kind="ExternalInput")
    out_d = nc.dram_tensor("out", [P, rows * 4], f16, kind="ExternalOutput")

    # const AP for the Exp bias (Exp needs an AP bias; Copy takes imm)
    cb = nc.alloc_sbuf_tensor("c_ln2", [P, 1], f32)
    nc.gpsimd.memset(cb.ap(), LN2)
    nc.const_aps.aps[(f32, LN2)] = cb.ap()
    gw = nc.alloc_sbuf_tensor("gp_warm", [P, 2], f32)
    nc.gpsimd.memset(gw.ap(), 1.0)
    nc.all_engine_barrier()

    with ExitStack() as ctx:
        def sb(name, w, dtype=f16):
            return ctx.enter_context(nc.sbuf_tensor(name, [P, w], dtype))

        zq = [sb(f"zq{j}", TK, i16) for j in range(2)]
        dth = [sb(f"dth{j}", TK) for j in range(2)]        # d, then th in place
        col = [sb(f"col{j}", TK * 3) for j in range(2)]
        qq2 = [sb(f"qq2{j}", TK) for j in range(2)]
        zdex = [sb(f"zdex{j}", TK) for j in range(2)]      # zd, then ex in place
        zmx4 = sb("zmx4", T * 4, i16)                      # lvl2 aliases [0:2T]
        zqmax = [sb(f"zqmax{j}", T) for j in range(2)]
        wcb = sb("wcb", TK * 4)                            # planes rgb + w2neg
        t4a = sb("t4a", T * 16)                            # lvl2 aliases [0:8T]
        qs4 = sb("qs4", T * 4)                             # lvl2 aliases [0:2T]
        prodq = [sb(f"prodq{j}", T) for j in range(2)]
        cw = [sb(f"cw{j}", T * 4) for j in range(2)]       # csumneg*3, wsumneg
        delta = [sb(f"delta{j}", T) for j in range(3)]
        denomn = sb("denomn", T, f32)
        rcpn = [sb(f"rcpn{j}", T, f32) for j in range(2)]
        ot = [sb(f"ot{j}", T * 4) for j in range(2)]       # planes r,g,b,a
        warm = sb("warm", 1, f32)

        s_inz = ctx.enter_context(nc.semaphore("s_inz"))
        s_ind = ctx.enter_context(nc.semaphore("s_ind"))
        s_inc = ctx.enter_context(nc.semaphore("s_inc"))
        s_out = [ctx.enter_context(nc.semaphore(f"s_out{j}")) for j in range(2)]
        s_act = ctx.enter_context(nc.semaphore("s_act"))
        s_dve = ctx.enter_context(nc.semaphore("s_dve"))
        s_gp = ctx.enter_context(nc.semaphore("s_gp"))

        marks = {}

        def mk(eng, name, t, c):
            marks[(eng, name, t)] = c

        def out_done(t):
            return 16 * (t // 2 + 1)

        # views
        def v_kt(buf):
            return buf[:].rearrange("p (k t) -> p k t", k=K)

        def v_ckt(buf):
            return buf[:].rearrange("p (c k t) -> p c k t", c=3, k=K)

        # ---------------- SP: input DMAs, double-buffered -----------------
        # col[t] is issued one iteration late so zq/d of the next phase
        # (needed first by DVE/ACT) are not queued behind a 3MB col xfer
        def sched_sp(sp):
            for t in range(n + 1):
                if sp is None:
                    continue
                if t < n:
                    if t >= 2:
                        sp.wait_ge(s_dve, marks[("d", "zd", t - 2)])
                    sp.dma_start(out=zq[t % 2][:], in_=zq_d[:, bass.ts(t, TK)]
                                 ).then_inc(s_inz, 16)
                    if t >= 2:
                        sp.wait_ge(s_dve, marks[("d", "w2", t - 2)])
                        sp.wait_ge(s_act, marks[("a", "qq2", t - 2)])
                    sp.dma_start(out=dth[t % 2][:], in_=ds_d[:, bass.ts(t, TK)]
                                 ).then_inc(s_ind, 16)
                tc = t - 1
                if 0 <= tc < n:
                    if tc >= 2:
                        sp.wait_ge(s_dve, marks[("d", "wc", tc - 2)])
                    sp.dma_start(out=col[tc % 2][:],
                                 in_=pc_d[:, bass.ts(tc, TK * 3)]
                                 ).then_inc(s_inc, 16)

        # ---------------- ACT ----------------
        def sched_act(act):
            c = 0
            if act is not None:
                act.activation(warm[:], warm[:], Act.Tanh, scale=1.0)
            for u in range(-1, n + 1):
                tz = u + 1
                if tz < n:
                    j = tz % 2
                    # th in place over d
                    if act is not None:
                        act.wait_ge(s_ind, 16 * (tz + 1))
                        act.activation(dth[j][:], dth[j][:], Act.Tanh,
                                       scale=1.0 / (2.0 * SIGMA)
                                       ).then_inc(s_act, 1)
                    c += 1
                    mk("a", "th", tz, c)
                    if act is not None:
                        if tz >= 2:
                            act.wait_ge(s_dve, marks[("d", "q1", tz - 2)])
                        act.activation(qq2[j][:], dth[j][:], Act.Copy,
                                       scale=1.0, bias=1.0).then_inc(s_act, 1)
                    c += 1
                    mk("a", "qq2", tz, c)
                    # ex in place over zd
                    if act is not None:
                        act.wait_ge(s_dve, marks[("d", "zd", tz)])
                        act.activation(zdex[j][:], zdex[j][:], Act.Exp,
                                       scale=S2).then_inc(s_act, 1)
                    c += 1
                    mk("a", "ex", tz, c)
                    if act is not None:
                        act.wait_ge(s_dve, marks[("d", "zqmax", tz)])
                        if tz >= 3:
                            act.wait_ge(s_dve, marks[("d", "denom", tz - 3)])
                            act.wait_ge(s_gp, marks[("g", "t3", tz - 3)])
                        act.activation(delta[tz % 3][:], zqmax[j][:], Act.Exp,
                                       scale=-S2, bias=LN2).then_inc(s_act, 1)
                    c += 1
                    mk("a", "delta", tz, c)
                ta = u
                if 0 <= ta < n:
                    if act is not None:
                        act.wait_ge(s_dve, marks[("d", "prodq", ta)])
                        if ta >= 2:
                            act.wait_ge(s_out[ta % 2], out_done(ta - 2))
                        ot_v = ot[ta % 2][:].rearrange("p (c t) -> p c t", c=4)
                        act.activation(ot_v[:, 3:4, :],
                                       prodq[ta % 2][:].unsqueeze(1),
                                       Act.Copy, scale=-1.0 / 256.0, bias=1.0
                                       ).then_inc(s_act, 1)
                    c += 1
                    mk("a", "alpha", ta, c)
                    if act is not None:
                        act.wait_ge(s_dve, marks[("d", "denom", ta)])
                        if ta >= 2:
                            act.wait_ge(s_gp, marks[("g", "rgb", ta - 2)])
                        # rcp = exp(-ln(denom)); Ln+Exp share the
                        # natural_log_exp_and_others table set (2 table
                        # loads per phase, ~2.6us ACT, saves 3.3us DVE)
                        act.activation(denomn[:], denomn[:], Act.Ln,
                                       scale=1.0)
                        act.activation(rcpn[ta % 2][:], denomn[:], Act.Exp,
                                       scale=-1.0).then_inc(s_act, 1)
                    c += 1
                    mk("a", "rcp", ta, c)
                to = u - 1
                if 0 <= to < n:
                    if act is not None:
                        act.wait_ge(s_gp, marks[("g", "rgb", to)])
                        act.dma_start(out=out_d[:, bass.ts(to, T * 4)],
                                      in_=ot[to % 2][:]).then_inc(s_out[to % 2], 16)
            if act is not None:
                act.wait_ge(s_out[0], 16 * ((n + 1) // 2))
                act.wait_ge(s_out[1], 16 * (n // 2))

        # ---------------- DVE ----------------
        def sched_dve(dve):
            c = 0
            for u in range(-1, n):
                tz = u + 1
                if 0 <= tz < n:
                    j = tz % 2
                    emit = dve is not None
                    if emit:
                        dve.wait_ge(s_inz, 16 * (tz + 1))
                        zq_v = v_kt(zq[j])
                        zx4 = zmx4[:].rearrange("p (k t) -> p k t", k=4)
                        dve.tensor_tensor(out=zx4, in0=zq_v[:, 0:4, :],
                                          in1=zq_v[:, 4:8, :], op=Alu.max)
                        dve.tensor_tensor(out=zx4[:, 0:2, :],
                                          in0=zx4[:, 0:2, :],
                                          in1=zx4[:, 2:4, :], op=Alu.max)
                        if tz >= 2:
                            dve.wait_ge(s_act, marks[("a", "delta", tz - 2)])
                        dve.tensor_tensor(out=zqmax[j][:].unsqueeze(1),
                                          in0=zx4[:, 0:1, :],
                                          in1=zx4[:, 1:2, :],
                                          op=Alu.max).then_inc(s_dve, 1)
                    c += 1
                    mk("d", "zqmax", tz, c)
                    if emit:
                        dve.tensor_tensor(
                            out=v_kt(zdex[j]),
                            in0=v_kt(zq[j]),
                            in1=zqmax[j][:].unsqueeze(1)
                                .broadcast_to((P, K, T)),
                            op=Alu.subtract).then_inc(s_dve, 1)
                    c += 1
                    mk("d", "zd", tz, c)
                t = u
                if not (0 <= t < n):
                    continue
                j = t % 2
                emit = dve is not None
                if emit:
                    dve.wait_ge(s_act, marks[("a", "qq2", t)])
                    q_v = v_kt(qq2[j])
                    q4 = qs4[:].rearrange("p (k t) -> p k t", k=4)
                    dve.tensor_tensor(out=q4, in0=q_v[:, 0:4, :],
                                      in1=q_v[:, 4:8, :],
                                      op=Alu.mult).then_inc(s_dve, 1)
                c += 1
                mk("d", "q1", t, c)
                if emit:
                    dve.tensor_tensor(out=q4[:, 0:2, :], in0=q4[:, 0:2, :],
                                      in1=q4[:, 2:4, :], op=Alu.mult)
                    if t >= 2:
                        dve.wait_ge(s_act, marks[("a", "alpha", t - 2)])
                    dve.tensor_tensor(out=prodq[j][:].unsqueeze(1),
                                      in0=q4[:, 0:1, :], in1=q4[:, 1:2, :],
                                      op=Alu.mult).then_inc(s_dve, 1)
                c += 1
                mk("d", "prodq", t, c)
                if emit:
                    dve.wait_ge(s_act, marks[("a", "ex", t)])
                    wcv = wcb[:].rearrange("p (c k t) -> p c k t", c=4, k=K)
                    dve.tensor_scalar(out=wcv[:, 3, :, :], in0=v_kt(dth[j]),
                                      scalar1=1.0, scalar2=-1.0,
                                      op0=Alu.subtract, op1=Alu.mult)
                    dve.tensor_tensor(out=wcv[:, 3, :, :],
                                      in0=wcv[:, 3, :, :],
                                      in1=v_kt(zdex[j]),
                                      op=Alu.mult).then_inc(s_dve, 1)
                c += 1
                mk("d", "w2", t, c)
                if emit:
                    dve.wait_ge(s_inc, 16 * (t + 1))
                    dve.tensor_tensor(
                        out=wcv[:, 0:3, :, :],
                        in0=wcv[:, 3:4, :, :].broadcast_to((P, 3, K, T)),
                        in1=v_ckt(col[j]),
                        op=Alu.mult).then_inc(s_dve, 1)
                c += 1
                mk("d", "wc", t, c)
                if emit:
                    t4 = t4a[:].rearrange("p (c k t) -> p c k t", c=4, k=4)
                    dve.tensor_tensor(out=t4, in0=wcv[:, :, 0:4, :],
                                      in1=wcv[:, :, 4:8, :], op=Alu.add)
                    dve.tensor_tensor(out=t4[:, :, 0:2, :],
                                      in0=t4[:, :, 0:2, :],
                                      in1=t4[:, :, 2:4, :], op=Alu.add)
                    if t >= 2:
                        dve.wait_ge(s_gp, marks[("g", "t3", t - 2)])
                    cw_v = cw[j][:].rearrange("p (c t) -> p c t", c=4)
                    dve.tensor_tensor(out=cw_v,
                                      in0=t4[:, :, 0, :],
                                      in1=t4[:, :, 1, :],
                                      op=Alu.add).then_inc(s_dve, 1)
                c += 1
                mk("d", "cw", t, c)
                if emit:
                    dve.wait_ge(s_act, marks[("a", "delta", t)])
                    if t >= 1:
                        dve.wait_ge(s_act, marks[("a", "rcp", t - 1)])
                    dve.scalar_tensor_tensor(
                        out=denomn[:], in0=cw_v[:, 3, :], scalar=1e-30,
                        in1=delta[t % 3][:], op0=Alu.max, op1=Alu.add,
                    ).then_inc(s_dve, 1)
                c += 1
                mk("d", "denom", t, c)

        # ---------------- GP ----------------
        def sched_gp(gp):
            c = 0
            if gp is not None:
                # warm the tensor_tensor ucode (~20us IRAM load) during
                # the initial DMA ramp, off the critical path
                gp.tensor_tensor(out=gw.ap(), in0=gw.ap(), in1=gw.ap(),
                                 op=Alu.add)
            for u in range(0, n + 1):
                to = u - 1
                if not (0 <= to < n):
                    continue
                jj = to % 2
                if gp is not None:
                    gp.wait_ge(s_dve, marks[("d", "cw", to)])
                    gp.wait_ge(s_act, marks[("a", "delta", to)])
                    if to + 1 < n:
                        # land in the next phase's denom/rcp window to limit
                        # SBUF-port contention with dense fp16 DVE ops
                        gp.wait_ge(s_dve, marks[("d", "denom", to + 1)])
                    if to >= 2:
                        gp.wait_ge(s_out[jj], out_done(to - 2))
                    cw_v = cw[jj][:].rearrange("p (c t) -> p c t", c=4)
                    ot_v = ot[jj][:].rearrange("p (c t) -> p c t", c=4)
                    gp.tensor_tensor(
                        out=ot_v[:, 0:3, :], in0=cw_v[:, 0:3, :],
                        in1=delta[to % 3][:].unsqueeze(1)
                            .broadcast_to((P, 3, T)),
                        op=Alu.add).then_inc(s_gp, 1)
                c += 1
                mk("g", "t3", to, c)
                if gp is not None:
                    gp.wait_ge(s_act, marks[("a", "rcp", to)])
                    gp.tensor_tensor(
                        out=ot_v[:, 0:3, :], in0=ot_v[:, 0:3, :],
                        in1=rcpn[jj][:].unsqueeze(1).broadcast_to((P, 3, T)),
                        op=Alu.mult).then_inc(s_gp, 1)
                c += 1
                mk("g", "rgb", to, c)

        sched_sp(None)
        sched_act(None)
        sched_dve(None)
        sched_gp(None)

        blk = ctx.enter_context(nc.Block())

        @blk.sync
        def _(sp):
            sched_sp(sp)

        @blk.scalar
        def _(act):
            sched_act(act)

        @blk.vector
        def _(dve):
            sched_dve(dve)

        @blk.gpsimd
        def _(gp):
            sched_gp(gp)

    return nc


_CACHE = {}


def _get_program(rows=2048, T=512):
    key = (rows, T)
    if key not in _CACHE:
        _CACHE[key] = build_program(rows, T)
    return _CACHE[key]


def _prep_core(zb, ds, pf, pc, rows, T):
    """Host-side repack for one core: returns dict of DRAM arrays."""
    n = rows // T
    mask = pf >= 0                                        # [P, rows, K]
    z_inv = (ZFAR - zb) * (1.0 / D)
    np.clip(z_inv, 0.0, 1.0, out=z_inv)
    zq = np.rint(z_inv * ZQ).astype(np.int16)
    zq[~mask] = 0
    d16 = ds.astype(np.float16)
    d16[~mask] = np.float16(30000.0)
    # k-major, phase-major: [P, rows, K] -> [P, n, K, T]
    zq = np.ascontiguousarray(
        zq.reshape(P, n, T, K).transpose(0, 1, 3, 2)).reshape(P, rows * K)
    d16 = np.ascontiguousarray(
        d16.reshape(P, n, T, K).transpose(0, 1, 3, 2)).reshape(P, rows * K)
    # colors: [P, rows, K, 3] -> [P, n, 3, K, T]
    c16 = pc.astype(np.float16).reshape(P, n, T, K, 3)
    c16 = np.ascontiguousarray(c16.transpose(0, 1, 4, 3, 2)
                               ).reshape(P, rows * 3 * K)
    return {"zq": zq, "dists": d16, "pixel_colors": c16}


def _run(pixel_colors, zbuf, dists, pix_to_face, trace=False):
    from concourse.bass_utils import run_bass_kernel_spmd

    N, H, W, Kk = zbuf.shape
    assert (N, H, W, Kk) == (8, 512, 512, 8), (N, H, W, Kk)
    rows = H * W // P  # 2048
    T = 512
    n = rows // T

    nc = _get_program(rows=rows, T=T)

    zb = np.asarray(zbuf, dtype=np.float32)
    ds = np.asarray(dists, dtype=np.float32)
    pf = np.asarray(pix_to_face)
    pc = np.asarray(pixel_colors, dtype=np.float32)

    in_maps = []
    for i in range(N_CORES):
        in_maps.append(_prep_core(
            zb[i].reshape(P, rows, K),
            ds[i].reshape(P, rows, K),
            pf[i].reshape(P, rows, K),
            pc[i].reshape(P, rows, K, 3),
            rows, T,
        ))

    res = run_bass_kernel_spmd(
        nc, in_maps, core_ids=list(range(N_CORES)), trace=trace
    )
    outs = []
    for i in range(N_CORES):
        o = res.results[i]["out"].astype(np.float32)
        # [P, n, 4, T] planar -> [P, rows, 4]
        o = o.reshape(P, n, 4, T).transpose(0, 1, 3, 2).reshape(H, W, 4)
        outs.append(o)
    return np.stack(outs, axis=0), res


def kernel(pixel_colors, zbuf, dists, pix_to_face):
    out, _ = _run(pixel_colors, zbuf, dists, pix_to_face, trace=False)
    return out


# revision 16
# speedup vs baseline: 1.2409x; 1.1171x over previous
"""Trainium2 Bass kernel for softmax RGB blend (pytorch3d NoLightShader).

Full inputs (N=8, H=512, W=512, K=8) are sharded batch-wise across 8
NeuronCores (one image per core); the blend is per-pixel so no cross-core
communication is needed.

Host-side input repack (per core, pure layout/dtype transforms):
  - mask folded into dists:  d' = masked ? +30000 : d   (fp16)
      (tanh(d'*5000) = 1  ->  q = 1, p = 0, exactly the masked case)
  - mask folded into z via quantization:
      zq = round((ZFAR - z)/D * 32767) * mask   (int16; 0 when masked,
      matching the reference's masked z_inv == 0 exactly)
  - colors fp16, k-major planar per phase:  [P, n, 3, K, T]
  - zq/d' k-major per phase: [P, n, K, T]
Output is written planar [P, n, 4, T] (r,g,b,a planes) and untransposed
on the host.

Math per pixel (K faces); everything is scaled by 256 (folded into pm2
and the delta bias) so that rcp = 1/denom' stays in fp16 range:
    th_k    = tanh(d_k*5000)          (ACT; masked -> 1)
    qq2_k   = 1 + th_k    (= 2q)      (ACT Copy;  prodq = prod_k qq2_k)
    alpha   = 1 - prodq/256           (ACT Copy, into out plane 3)
    zqmax   = max_k zq_k  (int16 TT-max tree -> f16; f16 rounding of
              zqmax is common-mode across k and cancels in the blend)
    zd_k    = zq_k - zqmax            (int16 - f16 bcast -> f16, 2x)
    ex_k    = exp(zd_k * S2)          (ACT, S2 = 1/(32767*GAMMA))
    pm2_k   = (th_k - 1)*(-256)       (DVE TS 4x, into wc plane 3)
    w2_k    = pm2_k * ex_k            (DVE TT in place, = 512*w_num)
    wc planes 0..2 = w2*col, plane 3 = w2; one 4-plane add-tree
              -> cw = [csum'(3), wsum'] (f16, all x256)
    delta'  = exp(-zqmax*S2 + ln(512)) (ACT; f16, = 512*delta_ref)
    denom'  = max(wsum',1e-27) + delta'  (DVE STT, f32)
    rcp'    = exp(-ln(denom'))        (ACT Ln+Exp -> f16; Ln and Exp
              share the natural_log_exp table set: 2 loads/phase)
    rcpc    = min(rcp', 60000)        (DVE TS 4x; overflow guard for
              the ~1e-5 of pixels with denom' below f16 range)
    t3'     = csum' + delta'          (DVE TT f16 2x, into out planes 0..2)
    rgb     = t3' * rcpc              (DVE TT f16 2x, in place)

All compute lives on DVE+ACT.  GPSIMD is intentionally unused: its SBUF
port is shared with the DVE, and measured contention slowed concurrent
dense fp16 DVE ops up to ~8x, wiping out any offload win.

Pipeline (phase u, n = rows/T phases): DVE iter u runs the z-stage for
u+1 first (zq tree, zd), then clamp/rgb for u-1 (hiding the ACT Ln/Exp
latency of phase u's rcp behind the z-stage), then the main stage for
u ending in denom and t3'.  ACT runs th/qq2/ex/delta for u+1, then
alpha/out-DMA/rcp.  SP streams input DMAs (double-buffered, col one
iteration behind zq/d so the next phase's small tensors aren't queued
behind a 3MB col transfer).
"""

import sys
from contextlib import ExitStack

import numpy as np

if "/opt/trn_rl_repo" not in sys.path:
    sys.path.insert(0, "/opt/trn_rl_repo")

SIGMA = 1e-4
GAMMA = 1e-4
ZNEAR = 1.0
ZFAR = 100.0
D = ZFAR - ZNEAR
ZQ = 32767.0                                   # z_inv quantization scale
S2 = 1.0 / (ZQ * GAMMA)                        # exp scale on zd
B_DELTA = float(np.log(512.0))                 # ln2 + ln(256) scaling

P = 128
K = 8
N_CORES = 8


def build_program(rows, T):
    import concourse.bass as bass
    from concourse import mybir

    dt = mybir.dt
    f32 = dt.float32
    f16 = dt.float16
    i16 = dt.int16
    Alu = mybir.AluOpType
    Act = mybir.ActivationFunctionType

    assert rows % T == 0
    n = rows // T
    TK = T * K

    nc = bass.Bass()

    zq_d = nc.dram_tensor("zq", [P, rows * K], i16, kind="ExternalInput")
    ds_d = nc.dram_tensor("dists", [P, rows * K], f16, kind="ExternalInput")
    pc_d = nc.dram_tensor("pixel_colors", [P, rows * 3 * K], f16,
                          kind="ExternalInput")
    out_d = nc.dram_tensor("out", [P, rows * 4], f16, kind="ExternalOutput")

    # const AP for the Exp bias (Exp needs an AP bias; Copy takes imm)
    cb = nc.alloc_sbuf_tensor("c_bd", [P, 1], f32)
    nc.gpsimd.memset(cb.ap(), B_DELTA)
    nc.const_aps.aps[(f32, B_DELTA)] = cb.ap()
    nc.all_engine_barrier()

    with ExitStack() as ctx:
        def sb(name, w, dtype=f16):
            return ctx.enter_context(nc.sbuf_tensor(name, [P, w], dtype))

        zq = [sb(f"zq{j}", TK, i16) for j in range(2)]
        dth = [sb(f"dth{j}", TK) for j in range(2)]        # d, then th in place
        col = [sb(f"col{j}", TK * 3) for j in range(2)]
        qq2 = [sb(f"qq2{j}", TK) for j in range(2)]
        zdex = [sb(f"zdex{j}", TK) for j in range(2)]      # zd, then ex in place
        zmx4 = sb("zmx4", T * 4, i16)                      # lvl2 aliases [0:2T]
        zqmax = [sb(f"zqmax{j}", T) for j in range(2)]
        wcb = sb("wcb", TK * 4)                            # planes rgb + w2
        t4a = sb("t4a", T * 16)                            # lvl2 aliases [0:8T]
        qs4 = sb("qs4", T * 4)                             # lvl2 aliases [0:2T]
        prodq = [sb(f"prodq{j}", T) for j in range(2)]
        cw = [sb(f"cw{j}", T * 4) for j in range(2)]       # csum'*3, wsum'
        delta = [sb(f"delta{j}", T) for j in range(3)]
        denomn = sb("denomn", T, f32)
        rcpn = [sb(f"rcpn{j}", T) for j in range(2)]       # from ACT
        rcpc = sb("rcpc", T)                               # clamped
        ot = [sb(f"ot{j}", T * 4) for j in range(2)]       # planes r,g,b,a
        warm = sb("warm", 1, f32)

        s_inz = ctx.enter_context(nc.semaphore("s_inz"))
        s_ind = ctx.enter_context(nc.semaphore("s_ind"))
        s_inc = ctx.enter_context(nc.semaphore("s_inc"))
        s_out = [ctx.enter_context(nc.semaphore(f"s_out{j}")) for j in range(2)]
        s_act = ctx.enter_context(nc.semaphore("s_act"))
        s_dve = ctx.enter_context(nc.semaphore("s_dve"))

        marks = {}

        def mk(eng, name, t, c):
            marks[(eng, name, t)] = c

        def out_done(t):
            return 16 * (t // 2 + 1)

        def v_kt(buf):
            return buf[:].rearrange("p (k t) -> p k t", k=K)

        def v_ckt(buf):
            return buf[:].rearrange("p (c k t) -> p c k t", c=3, k=K)

        # ---------------- SP: input DMAs, double-buffered -----------------
        # col[t] is issued one iteration late so zq/d of the next phase
        # (needed first by DVE/ACT) are not queued behind a 3MB col xfer
        def sched_sp(sp):
            for t in range(n + 1):
                if sp is None:
                    continue
                if t < n:
                    if t >= 2:
                        sp.wait_ge(s_dve, marks[("d", "zd", t - 2)])
                    sp.dma_start(out=zq[t % 2][:], in_=zq_d[:, bass.ts(t, TK)]
                                 ).then_inc(s_inz, 16)
                    if t >= 2:
                        sp.wait_ge(s_dve, marks[("d", "w2", t - 2)])
                        sp.wait_ge(s_act, marks[("a", "qq2", t - 2)])
                    sp.dma_start(out=dth[t % 2][:], in_=ds_d[:, bass.ts(t, TK)]
                                 ).then_inc(s_ind, 16)
                tc = t - 1
                if 0 <= tc < n:
                    if tc >= 2:
                        sp.wait_ge(s_dve, marks[("d", "wc", tc - 2)])
                    sp.dma_start(out=col[tc % 2][:],
                                 in_=pc_d[:, bass.ts(tc, TK * 3)]
                                 ).then_inc(s_inc, 16)

        # ---------------- ACT ----------------
        def sched_act(act):
            c = 0
            if act is not None:
                act.activation(warm[:], warm[:], Act.Tanh, scale=1.0)
            for u in range(-1, n + 1):
                tz = u + 1
                if tz < n:
                    j = tz % 2
                    # th in place over d
                    if act is not None:
                        act.wait_ge(s_ind, 16 * (tz + 1))
                        act.activation(dth[j][:], dth[j][:], Act.Tanh,
                                       scale=1.0 / (2.0 * SIGMA)
                                       ).then_inc(s_act, 1)
                    c += 1
                    mk("a", "th", tz, c)
                    if act is not None:
                        if tz >= 2:
                            act.wait_ge(s_dve, marks[("d", "q1", tz - 2)])
                        act.activation(qq2[j][:], dth[j][:], Act.Copy,
                                       scale=1.0, bias=1.0).then_inc(s_act, 1)
                    c += 1
                    mk("a", "qq2", tz, c)
                    # ex in place over zd
                    if act is not None:
                        act.wait_ge(s_dve, marks[("d", "zd", tz)])
                        act.activation(zdex[j][:], zdex[j][:], Act.Exp,
                                       scale=S2).then_inc(s_act, 1)
                    c += 1
                    mk("a", "ex", tz, c)
                    if act is not None:
                        act.wait_ge(s_dve, marks[("d", "zqmax", tz)])
                        if tz >= 3:
                            act.wait_ge(s_dve, marks[("d", "t3", tz - 3)])
                        act.activation(delta[tz % 3][:], zqmax[j][:], Act.Exp,
                                       scale=-S2, bias=B_DELTA
                                       ).then_inc(s_act, 1)
                    c += 1
                    mk("a", "delta", tz, c)
                ta = u
                if 0 <= ta < n:
                    if act is not None:
                        act.wait_ge(s_dve, marks[("d", "prodq", ta)])
                        if ta >= 2:
                            act.wait_ge(s_out[ta % 2], out_done(ta - 2))
                        ot_v = ot[ta % 2][:].rearrange("p (c t) -> p c t", c=4)
                        act.activation(ot_v[:, 3:4, :],
                                       prodq[ta % 2][:].unsqueeze(1),
                                       Act.Copy, scale=-1.0 / 256.0, bias=1.0
                                       ).then_inc(s_act, 1)
                    c += 1
                    mk("a", "alpha", ta, c)
                to = u - 1
                if 0 <= to < n:
                    if act is not None:
                        act.wait_ge(s_dve, marks[("d", "rgb", to)])
                        act.dma_start(out=out_d[:, bass.ts(to, T * 4)],
                                      in_=ot[to % 2][:]).then_inc(s_out[to % 2], 16)
                ta = u
                if 0 <= ta < n:
                    if act is not None:
                        act.wait_ge(s_dve, marks[("d", "denom", ta)])
                        # rcp' = exp(-ln(denom')) in f16; Ln+Exp share the
                        # natural_log_exp_and_others table set
                        act.activation(denomn[:], denomn[:], Act.Ln,
                                       scale=1.0)
                        act.activation(rcpn[ta % 2][:], denomn[:], Act.Exp,
                                       scale=-1.0).then_inc(s_act, 1)
                    c += 1
                    mk("a", "rcp", ta, c)
            if act is not None:
                act.wait_ge(s_out[0], 16 * ((n + 1) // 2))
                act.wait_ge(s_out[1], 16 * (n // 2))

        # ---------------- DVE ----------------
        def sched_dve(dve):
            c = 0
            for u in range(-1, n + 1):
                tz = u + 1
                if 0 <= tz < n:
                    j = tz % 2
                    emit = dve is not None
                    if emit:
                        dve.wait_ge(s_inz, 16 * (tz + 1))
                        zq_v = v_kt(zq[j])
                        zx4 = zmx4[:].rearrange("p (k t) -> p k t", k=4)
                        dve.tensor_tensor(out=zx4, in0=zq_v[:, 0:4, :],
                                          in1=zq_v[:, 4:8, :], op=Alu.max)
                        dve.tensor_tensor(out=zx4[:, 0:2, :],
                                          in0=zx4[:, 0:2, :],
                                          in1=zx4[:, 2:4, :], op=Alu.max)
                        if tz >= 2:
                            dve.wait_ge(s_act, marks[("a", "delta", tz - 2)])
                        dve.tensor_tensor(out=zqmax[j][:].unsqueeze(1),
                                          in0=zx4[:, 0:1, :],
                                          in1=zx4[:, 1:2, :],
                                          op=Alu.max).then_inc(s_dve, 1)
                    c += 1
                    mk("d", "zqmax", tz, c)
                    if emit:
                        dve.tensor_tensor(
                            out=v_kt(zdex[j]),
                            in0=v_kt(zq[j]),
                            in1=zqmax[j][:].unsqueeze(1)
                                .broadcast_to((P, K, T)),
                            op=Alu.subtract).then_inc(s_dve, 1)
                    c += 1
                    mk("d", "zd", tz, c)
                # clamp + rgb for u-1 (ACT's rcp latency hides behind the
                # z-stage above)
                tr = u - 1
                if 0 <= tr < n:
                    jr = tr % 2
                    if dve is not None:
                        dve.wait_ge(s_act, marks[("a", "rcp", tr)])
                        dve.tensor_scalar_min(rcpc[:], rcpn[jr][:], 60000.0)
                        otr_v = ot[jr][:].rearrange("p (c t) -> p c t", c=4)
                        dve.tensor_tensor(
                            out=otr_v[:, 0:3, :], in0=otr_v[:, 0:3, :],
                            in1=rcpc[:].unsqueeze(1).broadcast_to((P, 3, T)),
                            op=Alu.mult).then_inc(s_dve, 1)
                    c += 1
                    mk("d", "rgb", tr, c)
                t = u
                if not (0 <= t < n):
                    continue
                j = t % 2
                emit = dve is not None
                if emit:
                    dve.wait_ge(s_act, marks[("a", "qq2", t)])
                    q_v = v_kt(qq2[j])
                    q4 = qs4[:].rearrange("p (k t) -> p k t", k=4)
                    dve.tensor_tensor(out=q4, in0=q_v[:, 0:4, :],
                                      in1=q_v[:, 4:8, :],
                                      op=Alu.mult).then_inc(s_dve, 1)
                c += 1
                mk("d", "q1", t, c)
                if emit:
                    dve.tensor_tensor(out=q4[:, 0:2, :], in0=q4[:, 0:2, :],
                                      in1=q4[:, 2:4, :], op=Alu.mult)
                    if t >= 2:
                        dve.wait_ge(s_act, marks[("a", "alpha", t - 2)])
                    dve.tensor_tensor(out=prodq[j][:].unsqueeze(1),
                                      in0=q4[:, 0:1, :], in1=q4[:, 1:2, :],
                                      op=Alu.mult).then_inc(s_dve, 1)
                c += 1
                mk("d", "prodq", t, c)
                if emit:
                    dve.wait_ge(s_act, marks[("a", "ex", t)])
                    wcv = wcb[:].rearrange("p (c k t) -> p c k t", c=4, k=K)
                    dve.tensor_scalar(out=wcv[:, 3, :, :], in0=v_kt(dth[j]),
                                      scalar1=1.0, scalar2=-256.0,
                                      op0=Alu.subtract, op1=Alu.mult)
                    dve.tensor_tensor(out=wcv[:, 3, :, :],
                                      in0=wcv[:, 3, :, :],
                                      in1=v_kt(zdex[j]),
                                      op=Alu.mult).then_inc(s_dve, 1)
                c += 1
                mk("d", "w2", t, c)
                if emit:
                    dve.wait_ge(s_inc, 16 * (t + 1))
                    dve.tensor_tensor(
                        out=wcv[:, 0:3, :, :],
                        in0=wcv[:, 3:4, :, :].broadcast_to((P, 3, K, T)),
                        in1=v_ckt(col[j]),
                        op=Alu.mult).then_inc(s_dve, 1)
                c += 1
                mk("d", "wc", t, c)
                if emit:
                    t4 = t4a[:].rearrange("p (c k t) -> p c k t", c=4, k=4)
                    dve.tensor_tensor(out=t4, in0=wcv[:, :, 0:4, :],
                                      in1=wcv[:, :, 4:8, :], op=Alu.add)
                    dve.tensor_tensor(out=t4[:, :, 0:2, :],
                                      in0=t4[:, :, 0:2, :],
                                      in1=t4[:, :, 2:4, :], op=Alu.add)
                    cw_v = cw[j][:].rearrange("p (c t) -> p c t", c=4)
                    dve.tensor_tensor(out=cw_v,
                                      in0=t4[:, :, 0, :],
                                      in1=t4[:, :, 1, :],
                                      op=Alu.add).then_inc(s_dve, 1)
                c += 1
                mk("d", "cw", t, c)
                if emit:
                    dve.wait_ge(s_act, marks[("a", "delta", t)])
                    if t >= 1:
                        dve.wait_ge(s_act, marks[("a", "rcp", t - 1)])
                    dve.scalar_tensor_tensor(
                        out=denomn[:], in0=cw_v[:, 3, :], scalar=1e-27,
                        in1=delta[t % 3][:], op0=Alu.max, op1=Alu.add,
                    ).then_inc(s_dve, 1)
                c += 1
                mk("d", "denom", t, c)
                if emit:
                    if t >= 2:
                        dve.wait_ge(s_out[j], out_done(t - 2))
                    ot_v = ot[j][:].rearrange("p (c t) -> p c t", c=4)
                    dve.tensor_tensor(
                        out=ot_v[:, 0:3, :], in0=cw_v[:, 0:3, :],
                        in1=delta[t % 3][:].unsqueeze(1)
                            .broadcast_to((P, 3, T)),
                        op=Alu.add).then_inc(s_dve, 1)
                c += 1
                mk("d", "t3", t, c)

        sched_sp(None)
        sched_act(None)
        sched_dve(None)

        blk = ctx.enter_context(nc.Block())

        @blk.sync
        def _(sp):
            sched_sp(sp)

        @blk.scalar
        def _(act):
            sched_act(act)

        @blk.vector
        def _(dve):
            sched_dve(dve)

    return nc


_CACHE = {}


def _get_program(rows=2048, T=512):
    key = (rows, T)
    if key not in _CACHE:
        _CACHE[key] = build_program(rows, T)
    return _CACHE[key]


def _prep_core(zb, ds, pf, pc, rows, T):
    """Host-side repack for one core: returns dict of DRAM arrays."""
    n = rows // T
    mask = pf >= 0                                        # [P, rows, K]
    z_inv = (ZFAR - zb) * (1.0 / D)
    np.clip(z_inv, 0.0, 1.0, out=z_inv)
    zq = np.rint(z_inv * ZQ).astype(np.int16)
    zq[~mask] = 0
    d16 = ds.astype(np.float16)
    d16[~mask] = np.float16(30000.0)
    # k-major, phase-major: [P, rows, K] -> [P, n, K, T]
    zq = np.ascontiguousarray(
        zq.reshape(P, n, T, K).transpose(0, 1, 3, 2)).reshape(P, rows * K)
    d16 = np.ascontiguousarray(
        d16.reshape(P, n, T, K).transpose(0, 1, 3, 2)).reshape(P, rows * K)
    # colors: [P, rows, K, 3] -> [P, n, 3, K, T]
    c16 = pc.astype(np.float16).reshape(P, n, T, K, 3)
    c16 = np.ascontiguousarray(c16.transpose(0, 1, 4, 3, 2)
                               ).reshape(P, rows * 3 * K)
    return {"zq": zq, "dists": d16, "pixel_colors": c16}


def _run(pixel_colors, zbuf, dists, pix_to_face, trace=False):
    from concourse.bass_utils import run_bass_kernel_spmd

    N, H, W, Kk = zbuf.shape
    assert (N, H, W, Kk) == (8, 512, 512, 8), (N, H, W, Kk)
    rows = H * W // P  # 2048
    T = 512
    n = rows // T

    nc = _get_program(rows=rows, T=T)

    zb = np.asarray(zbuf, dtype=np.float32)
    ds = np.asarray(dists, dtype=np.float32)
    pf = np.asarray(pix_to_face)
    pc = np.asarray(pixel_colors, dtype=np.float32)

    in_maps = []
    for i in range(N_CORES):
        in_maps.append(_prep_core(
            zb[i].reshape(P, rows, K),
            ds[i].reshape(P, rows, K),
            pf[i].reshape(P, rows, K),
            pc[i].reshape(P, rows, K, 3),
            rows, T,
        ))

    res = run_bass_kernel_spmd(
        nc, in_maps, core_ids=list(range(N_CORES)), trace=trace
    )
    outs = []
    for i in range(N_CORES):
        o = res.results[i]["out"].astype(np.float32)
        o = o.reshape(P, n, 4, T).transpose(0, 1, 3, 2).reshape(H, W, 4)
        outs.append(o)
    return np.stack(outs, axis=0), res


def kernel(pixel_colors, zbuf, dists, pix_to_face):
    out, _ = _run(pixel_colors, zbuf, dists, pix_to_face, trace=False)
    return out


# revision 21
# speedup vs baseline: 1.2923x; 1.0414x over previous
"""Trainium2 Bass kernel for softmax RGB blend (pytorch3d NoLightShader).

Full inputs (N=8, H=512, W=512, K=8) are sharded batch-wise across 8
NeuronCores (one image per core); the blend is per-pixel so no cross-core
communication is needed.

Host-side input repack (per core, pure layout/dtype transforms):
  - mask folded into dists:  d' = masked ? +30000 : d   (fp16)
      (tanh(d'*5000) = 1  ->  q = 1, p = 0, exactly the masked case)
  - mask folded into z via quantization:
      zq = round((ZFAR - z)/D * 32767) * mask   (int16; 0 when masked,
      matching the reference's masked z_inv == 0 exactly)
  - colors fp16, k-major planar per phase:  [P, n, 3, K, T]
  - zq/d' k-major per phase: [P, n, K, T]
Output is written planar [P, n, 4, T] (r,g,b,a planes) and untransposed
on the host.

Math per pixel (K faces); everything is scaled by 256 (folded into pm2
and the delta bias) so that rcp = 1/denom' stays in fp16 range:
    th_k    = tanh(d_k*5000)          (ACT; masked -> 1)
    qq2_k   = 1 + th_k    (= 2q)      (ACT Copy;  prodq = prod_k qq2_k)
    alpha   = 1 - prodq/256           (ACT Copy, into out plane 3)
    zqmax   = max_k zq_k  (int16 TT-max tree -> f16; f16 rounding of
              zqmax is common-mode across k and cancels in the blend)
    zd_k    = zq_k - zqmax            (int16 - f16 bcast -> f16, 2x)
    ex_k    = exp(zd_k * S2)          (ACT, S2 = 1/(32767*GAMMA))
    pm2_k   = (th_k - 1)*(-256)       (DVE TS 4x, into wc plane 3)
    w2_k    = pm2_k * ex_k            (DVE TT in place, = 512*w_num)
    wc planes 0..2 = w2*col, plane 3 = w2; one 4-plane add-tree
              -> cw = [csum'(3), wsum'] (f16, all x256)
    delta'  = exp(-zqmax*S2 + ln(512)) (ACT; f16, = 512*delta_ref)
    denom'  = max(wsum',1e-27) + delta'  (DVE STT, f32)
    rcp'    = exp(-ln(denom'))        (ACT Ln+Exp -> f16; Ln and Exp
              share the natural_log_exp table set: 2 loads/phase)
    rcpc    = min(rcp', 60000)        (DVE TS 4x; overflow guard for
              the ~1e-5 of pixels with denom' below f16 range)
    t3'     = csum' + delta'          (DVE TT f16 2x, into out planes 0..2)
    rgb     = t3' * rcpc              (DVE TT f16 2x, in place)

All compute lives on DVE+ACT.  GPSIMD is intentionally unused: its SBUF
port is shared with the DVE, and measured contention slowed concurrent
dense fp16 DVE ops up to ~8x, wiping out any offload win.

Pipeline (phase u, n = rows/T phases): DVE iter u runs the z-stage for
u+1 first (zq tree, zd), then clamp/rgb for u-1 (hiding the ACT Ln/Exp
latency of phase u's rcp behind the z-stage), then the main stage for
u ending in denom and t3'.  ACT runs th/qq2/ex/delta for u+1, then
alpha/out-DMA/rcp.  SP streams input DMAs (double-buffered, col one
iteration behind zq/d so the next phase's small tensors aren't queued
behind a 3MB col transfer).
"""

import sys
from contextlib import ExitStack

import numpy as np

if "/opt/trn_rl_repo" not in sys.path:
    sys.path.insert(0, "/opt/trn_rl_repo")

SIGMA = 1e-4
GAMMA = 1e-4
ZNEAR = 1.0
ZFAR = 100.0
D = ZFAR - ZNEAR
ZQ = 32767.0                                   # z_inv quantization scale
S2 = 1.0 / (ZQ * GAMMA)                        # exp scale on zd
B_DELTA = float(np.log(512.0))                 # ln2 + ln(256) scaling

P = 128
K = 8
N_CORES = 8


def build_program(rows, TS):
    """TS: list of per-phase tile sizes (pixels per partition), sum == rows.
    Small first/last phases shorten pipeline fill/drain."""
    import concourse.bass as bass
    from concourse import mybir

    dt = mybir.dt
    f32 = dt.float32
    f16 = dt.float16
    i16 = dt.int16
    Alu = mybir.AluOpType
    Act = mybir.ActivationFunctionType

    assert sum(TS) == rows
    n = len(TS)
    off = [sum(TS[:t]) for t in range(n)]      # row offset of each phase
    T = max(TS)
    TK = T * K

    nc = bass.Bass()

    zq_d = nc.dram_tensor("zq", [P, rows * K], i16, kind="ExternalInput")
    ds_d = nc.dram_tensor("dists", [P, rows * K], f16, kind="ExternalInput")
    pc_d = nc.dram_tensor("pixel_colors", [P, rows * 3 * K], f16,
                          kind="ExternalInput")
    out_d = nc.dram_tensor("out", [P, rows * 4], f16, kind="ExternalOutput")

    # const AP for the Exp bias (Exp needs an AP bias; Copy takes imm)
    cb = nc.alloc_sbuf_tensor("c_bd", [P, 1], f32)
    nc.gpsimd.memset(cb.ap(), B_DELTA)
    nc.const_aps.aps[(f32, B_DELTA)] = cb.ap()
    nc.all_engine_barrier()

    with ExitStack() as ctx:
        def sb(name, w, dtype=f16):
            return ctx.enter_context(nc.sbuf_tensor(name, [P, w], dtype))

        zq = [sb(f"zq{j}", TK, i16) for j in range(2)]
        dth = [sb(f"dth{j}", TK) for j in range(2)]        # d, then th in place
        col = [sb(f"col{j}", TK * 3) for j in range(2)]
        qq2 = [sb(f"qq2{j}", TK) for j in range(2)]
        zdex = [sb(f"zdex{j}", TK) for j in range(2)]      # zd, then ex in place
        zmx4 = sb("zmx4", T * 4, i16)                      # lvl2 aliases [0:2T]
        zqmax = [sb(f"zqmax{j}", T) for j in range(2)]
        wcb = sb("wcb", TK * 4)                            # planes rgb + w2
        t4a = sb("t4a", T * 16)                            # lvl2 aliases [0:8T]
        qs4 = sb("qs4", T * 4)                             # lvl2 aliases [0:2T]
        prodq = [sb(f"prodq{j}", T) for j in range(2)]
        cw = [sb(f"cw{j}", T * 4) for j in range(2)]       # csum'*3, wsum'
        delta = [sb(f"delta{j}", T) for j in range(3)]
        denomn = sb("denomn", T, f32)
        rcpn = [sb(f"rcpn{j}", T) for j in range(2)]       # from ACT
        rcpc = sb("rcpc", T)                               # clamped
        ot = [sb(f"ot{j}", T * 4) for j in range(2)]       # planes r,g,b,a
        warm = sb("warm", 1, f32)

        s_inz = ctx.enter_context(nc.semaphore("s_inz"))
        s_ind = ctx.enter_context(nc.semaphore("s_ind"))
        s_inc = ctx.enter_context(nc.semaphore("s_inc"))
        s_out = [ctx.enter_context(nc.semaphore(f"s_out{j}")) for j in range(2)]
        s_act = ctx.enter_context(nc.semaphore("s_act"))
        s_dve = ctx.enter_context(nc.semaphore("s_dve"))

        marks = {}

        def mk(eng, name, t, c):
            marks[(eng, name, t)] = c

        def out_done(t):
            return 16 * (t // 2 + 1)

        def v_kt(buf, Tl):
            return buf[:, 0:K * Tl].rearrange("p (k t) -> p k t", k=K)

        def v_ckt(buf, Tl):
            return buf[:, 0:3 * K * Tl].rearrange("p (c k t) -> p c k t",
                                                  c=3, k=K)

        # ---------------- SP: input DMAs, double-buffered -----------------
        # col[t] is issued one iteration late so zq/d of the next phase
        # (needed first by DVE/ACT) are not queued behind a 3MB col xfer
        def sched_sp(sp):
            for t in range(n + 1):
                if sp is None:
                    continue
                if t < n:
                    o, Tl = off[t], TS[t]
                    if t >= 2:
                        sp.wait_ge(s_dve, marks[("d", "zd", t - 2)])
                    sp.dma_start(out=zq[t % 2][:, 0:K * Tl],
                                 in_=zq_d[:, o * K:(o + Tl) * K]
                                 ).then_inc(s_inz, 16)
                    if t >= 2:
                        sp.wait_ge(s_dve, marks[("d", "w2", t - 2)])
                        sp.wait_ge(s_act, marks[("a", "qq2", t - 2)])
                    sp.dma_start(out=dth[t % 2][:, 0:K * Tl],
                                 in_=ds_d[:, o * K:(o + Tl) * K]
                                 ).then_inc(s_ind, 16)
                tc = t - 1
                if 0 <= tc < n:
                    o, Tl = off[tc], TS[tc]
                    if tc >= 2:
                        sp.wait_ge(s_dve, marks[("d", "wc", tc - 2)])
                    sp.dma_start(out=col[tc % 2][:, 0:3 * K * Tl],
                                 in_=pc_d[:, o * 3 * K:(o + Tl) * 3 * K]
                                 ).then_inc(s_inc, 16)

        # ---------------- ACT ----------------
        def sched_act(act):
            c = 0
            if act is not None:
                act.activation(warm[:], warm[:], Act.Tanh, scale=1.0)
            for u in range(-1, n + 1):
                tz = u + 1
                if tz < n:
                    j = tz % 2
                    Tl = TS[tz]
                    # th in place over d
                    if act is not None:
                        act.wait_ge(s_ind, 16 * (tz + 1))
                        act.activation(dth[j][:, 0:K * Tl],
                                       dth[j][:, 0:K * Tl], Act.Tanh,
                                       scale=1.0 / (2.0 * SIGMA)
                                       ).then_inc(s_act, 1)
                    c += 1
                    mk("a", "th", tz, c)
                    if act is not None:
                        if tz >= 2:
                            act.wait_ge(s_dve, marks[("d", "q1", tz - 2)])
                        act.activation(qq2[j][:, 0:K * Tl],
                                       dth[j][:, 0:K * Tl], Act.Copy,
                                       scale=1.0, bias=1.0).then_inc(s_act, 1)
                    c += 1
                    mk("a", "qq2", tz, c)
                    # ex in place over zd
                    if act is not None:
                        act.wait_ge(s_dve, marks[("d", "zd", tz)])
                        act.activation(zdex[j][:, 0:K * Tl],
                                       zdex[j][:, 0:K * Tl], Act.Exp,
                                       scale=S2).then_inc(s_act, 1)
                    c += 1
                    mk("a", "ex", tz, c)
                    if act is not None:
                        act.wait_ge(s_dve, marks[("d", "zqmax", tz)])
                        if tz >= 3:
                            act.wait_ge(s_dve, marks[("d", "t3", tz - 3)])
                        act.activation(delta[tz % 3][:, 0:Tl],
                                       zqmax[j][:, 0:Tl], Act.Exp,
                                       scale=-S2, bias=B_DELTA
                                       ).then_inc(s_act, 1)
                    c += 1
                    mk("a", "delta", tz, c)
                ta = u
                if 0 <= ta < n:
                    Tl = TS[ta]
                    if act is not None:
                        act.wait_ge(s_dve, marks[("d", "prodq", ta)])
                        if ta >= 2:
                            act.wait_ge(s_out[ta % 2], out_done(ta - 2))
                        ot_v = ot[ta % 2][:, 0:4 * Tl].rearrange(
                            "p (c t) -> p c t", c=4)
                        act.activation(ot_v[:, 3:4, :],
                                       prodq[ta % 2][:, 0:Tl].unsqueeze(1),
                                       Act.Copy, scale=-1.0 / 256.0, bias=1.0
                                       ).then_inc(s_act, 1)
                    c += 1
                    mk("a", "alpha", ta, c)
                to = u - 1
                if 0 <= to < n:
                    if act is not None:
                        oo, Tl = off[to], TS[to]
                        act.wait_ge(s_dve, marks[("d", "rgb", to)])
                        act.dma_start(
                            out=out_d[:, oo * 4:(oo + Tl) * 4],
                            in_=ot[to % 2][:, 0:4 * Tl]
                        ).then_inc(s_out[to % 2], 16)
                ta = u
                if 0 <= ta < n:
                    Tl = TS[ta]
                    if act is not None:
                        act.wait_ge(s_dve, marks[("d", "denom", ta)])
                        # rcp' = exp(-ln(denom')) in f16; Ln+Exp share the
                        # natural_log_exp_and_others table set
                        act.activation(denomn[:, 0:Tl], denomn[:, 0:Tl],
                                       Act.Ln, scale=1.0)
                        act.activation(rcpn[ta % 2][:, 0:Tl],
                                       denomn[:, 0:Tl], Act.Exp,
                                       scale=-1.0).then_inc(s_act, 1)
                    c += 1
                    mk("a", "rcp", ta, c)
            if act is not None:
                act.wait_ge(s_out[0], 16 * ((n + 1) // 2))
                act.wait_ge(s_out[1], 16 * (n // 2))

        # ---------------- DVE ----------------
        def sched_dve(dve):
            c = 0
            for u in range(-1, n + 1):
                tz = u + 1
                if 0 <= tz < n:
                    j = tz % 2
                    Tl = TS[tz]
                    emit = dve is not None
                    if emit:
                        dve.wait_ge(s_inz, 16 * (tz + 1))
                        zq_v = v_kt(zq[j], Tl)
                        zx4 = zmx4[:, 0:4 * Tl].rearrange(
                            "p (k t) -> p k t", k=4)
                        dve.tensor_tensor(out=zx4, in0=zq_v[:, 0:4, :],
                                          in1=zq_v[:, 4:8, :], op=Alu.max)
                        dve.tensor_tensor(out=zx4[:, 0:2, :],
                                          in0=zx4[:, 0:2, :],
                                          in1=zx4[:, 2:4, :], op=Alu.max)
                        if tz >= 2:
                            dve.wait_ge(s_act, marks[("a", "delta", tz - 2)])
                        dve.tensor_tensor(out=zqmax[j][:, 0:Tl].unsqueeze(1),
                                          in0=zx4[:, 0:1, :],
                                          in1=zx4[:, 1:2, :],
                                          op=Alu.max).then_inc(s_dve, 1)
                    c += 1
                    mk("d", "zqmax", tz, c)
                    if emit:
                        dve.tensor_tensor(
                            out=v_kt(zdex[j], Tl),
                            in0=v_kt(zq[j], Tl),
                            in1=zqmax[j][:, 0:Tl].unsqueeze(1)
                                .broadcast_to((P, K, Tl)),
                            op=Alu.subtract).then_inc(s_dve, 1)
                    c += 1
                    mk("d", "zd", tz, c)
                # clamp + rgb for u-1 (ACT's rcp latency hides behind the
                # z-stage above)
                tr = u - 1
                if 0 <= tr < n:
                    jr = tr % 2
                    Tl = TS[tr]
                    if dve is not None:
                        dve.wait_ge(s_act, marks[("a", "rcp", tr)])
                        dve.tensor_scalar_min(rcpc[:, 0:Tl],
                                              rcpn[jr][:, 0:Tl], 60000.0)
                        otr_v = ot[jr][:, 0:4 * Tl].rearrange(
                            "p (c t) -> p c t", c=4)
                        dve.tensor_tensor(
                            out=otr_v[:, 0:3, :], in0=otr_v[:, 0:3, :],
                            in1=rcpc[:, 0:Tl].unsqueeze(1)
                                .broadcast_to((P, 3, Tl)),
                            op=Alu.mult).then_inc(s_dve, 1)
                    c += 1
                    mk("d", "rgb", tr, c)
                t = u
                if not (0 <= t < n):
                    continue
                j = t % 2
                Tl = TS[t]
                emit = dve is not None
                if emit:
                    dve.wait_ge(s_act, marks[("a", "qq2", t)])
                    q_v = v_kt(qq2[j], Tl)
                    q4 = qs4[:, 0:4 * Tl].rearrange("p (k t) -> p k t", k=4)
                    dve.tensor_tensor(out=q4, in0=q_v[:, 0:4, :],
                                      in1=q_v[:, 4:8, :],
                                      op=Alu.mult).then_inc(s_dve, 1)
                c += 1
                mk("d", "q1", t, c)
                if emit:
                    dve.tensor_tensor(out=q4[:, 0:2, :], in0=q4[:, 0:2, :],
                                      in1=q4[:, 2:4, :], op=Alu.mult)
                    if t >= 2:
                        dve.wait_ge(s_act, marks[("a", "alpha", t - 2)])
                    dve.tensor_tensor(out=prodq[j][:, 0:Tl].unsqueeze(1),
                                      in0=q4[:, 0:1, :], in1=q4[:, 1:2, :],
                                      op=Alu.mult).then_inc(s_dve, 1)
                c += 1
                mk("d", "prodq", t, c)
                if emit:
                    dve.wait_ge(s_act, marks[("a", "ex", t)])
                    wcv = wcb[:, 0:4 * K * Tl].rearrange(
                        "p (c k t) -> p c k t", c=4, k=K)
                    dve.tensor_scalar(out=wcv[:, 3, :, :],
                                      in0=v_kt(dth[j], Tl),
                                      scalar1=1.0, scalar2=-256.0,
                                      op0=Alu.subtract, op1=Alu.mult)
                    dve.tensor_tensor(out=wcv[:, 3, :, :],
                                      in0=wcv[:, 3, :, :],
                                      in1=v_kt(zdex[j], Tl),
                                      op=Alu.mult).then_inc(s_dve, 1)
                c += 1
                mk("d", "w2", t, c)
                if emit:
                    dve.wait_ge(s_inc, 16 * (t + 1))
                    dve.tensor_tensor(
                        out=wcv[:, 0:3, :, :],
                        in0=wcv[:, 3:4, :, :].broadcast_to((P, 3, K, Tl)),
                        in1=v_ckt(col[j], Tl),
                        op=Alu.mult).then_inc(s_dve, 1)
                c += 1
                mk("d", "wc", t, c)
                if emit:
                    t4 = t4a[:, 0:16 * Tl].rearrange(
                        "p (c k t) -> p c k t", c=4, k=4)
                    dve.tensor_tensor(out=t4, in0=wcv[:, :, 0:4, :],
                                      in1=wcv[:, :, 4:8, :], op=Alu.add)
                    dve.tensor_tensor(out=t4[:, :, 0:2, :],
                                      in0=t4[:, :, 0:2, :],
                                      in1=t4[:, :, 2:4, :], op=Alu.add)
                    cw_v = cw[j][:, 0:4 * Tl].rearrange(
                        "p (c t) -> p c t", c=4)
                    dve.tensor_tensor(out=cw_v,
                                      in0=t4[:, :, 0, :],
                                      in1=t4[:, :, 1, :],
                                      op=Alu.add).then_inc(s_dve, 1)
                c += 1
                mk("d", "cw", t, c)
                if emit:
                    dve.wait_ge(s_act, marks[("a", "delta", t)])
                    if t >= 1:
                        dve.wait_ge(s_act, marks[("a", "rcp", t - 1)])
                    dve.scalar_tensor_tensor(
                        out=denomn[:, 0:Tl], in0=cw_v[:, 3, :], scalar=1e-27,
                        in1=delta[t % 3][:, 0:Tl], op0=Alu.max, op1=Alu.add,
                    ).then_inc(s_dve, 1)
                c += 1
                mk("d", "denom", t, c)
                if emit:
                    if t >= 2:
                        dve.wait_ge(s_out[j], out_done(t - 2))
                    ot_v = ot[j][:, 0:4 * Tl].rearrange(
                        "p (c t) -> p c t", c=4)
                    dve.tensor_tensor(
                        out=ot_v[:, 0:3, :], in0=cw_v[:, 0:3, :],
                        in1=delta[t % 3][:, 0:Tl].unsqueeze(1)
                            .broadcast_to((P, 3, Tl)),
                        op=Alu.add).then_inc(s_dve, 1)
                c += 1
                mk("d", "t3", t, c)

        sched_sp(None)
        sched_act(None)
        sched_dve(None)

        blk = ctx.enter_context(nc.Block())

        @blk.sync
        def _(sp):
            sched_sp(sp)

        @blk.scalar
        def _(act):
            sched_act(act)

        @blk.vector
        def _(dve):
            sched_dve(dve)

    return nc


_CACHE = {}

# small first/last phases shorten pipeline fill/drain
TS_PHASES = (256, 512, 512, 512, 256)


def _get_program(rows=2048, TS=TS_PHASES):
    key = (rows, TS)
    if key not in _CACHE:
        _CACHE[key] = build_program(rows, list(TS))
    return _CACHE[key]


def _kmaj(a, TS, inner):
    """[P, rows, K, inner...] -> per-phase k-major planar, flattened."""
    parts = []
    o = 0
    for Tl in TS:
        s = a[:, o:o + Tl]                     # [P, Tl, K] or [P, Tl, K, 3]
        if s.ndim == 3:
            s = s.transpose(0, 2, 1)           # [P, K, Tl]
        else:
            s = s.transpose(0, 3, 2, 1)        # [P, 3, K, Tl]
        parts.append(np.ascontiguousarray(s).reshape(P, -1))
        o += Tl
    return np.concatenate(parts, axis=1)


def _prep_core(zb, ds, pf, pc, TS):
    """Host-side repack for one core: returns dict of DRAM arrays."""
    mask = pf >= 0                                        # [P, rows, K]
    z_inv = (ZFAR - zb) * (1.0 / D)
    np.clip(z_inv, 0.0, 1.0, out=z_inv)
    zq = np.rint(z_inv * ZQ).astype(np.int16)
    zq[~mask] = 0
    d16 = ds.astype(np.float16)
    d16[~mask] = np.float16(30000.0)
    return {
        "zq": _kmaj(zq, TS, 1),
        "dists": _kmaj(d16, TS, 1),
        "pixel_colors": _kmaj(pc.astype(np.float16), TS, 3),
    }


def _run(pixel_colors, zbuf, dists, pix_to_face, trace=False):
    from concourse.bass_utils import run_bass_kernel_spmd

    N, H, W, Kk = zbuf.shape
    assert (N, H, W, Kk) == (8, 512, 512, 8), (N, H, W, Kk)
    rows = H * W // P  # 2048
    TS = TS_PHASES
    assert sum(TS) == rows

    nc = _get_program(rows=rows, TS=TS)

    zb = np.asarray(zbuf, dtype=np.float32)
    ds = np.asarray(dists, dtype=np.float32)
    pf = np.asarray(pix_to_face)
    pc = np.asarray(pixel_colors, dtype=np.float32)

    in_maps = []
    for i in range(N_CORES):
        in_maps.append(_prep_core(
            zb[i].reshape(P, rows, K),
            ds[i].reshape(P, rows, K),
            pf[i].reshape(P, rows, K),
            pc[i].reshape(P, rows, K, 3),
            TS,
        ))

    res = run_bass_kernel_spmd(
        nc, in_maps, core_ids=list(range(N_CORES)), trace=trace
    )
    outs = []
    for i in range(N_CORES):
        o = res.results[i]["out"].astype(np.float32)
        # per-phase planar [P, 4, Tl] -> [P, rows, 4]
        parts = []
        oo = 0
        for Tl in TS:
            chunk = o[:, oo * 4:(oo + Tl) * 4].reshape(P, 4, Tl)
            parts.append(chunk.transpose(0, 2, 1))
            oo += Tl
        full = np.concatenate(parts, axis=1).reshape(H, W, 4)
        outs.append(full)
    return np.stack(outs, axis=0), res


def kernel(pixel_colors, zbuf, dists, pix_to_face):
    out, _ = _run(pixel_colors, zbuf, dists, pix_to_face, trace=False)
    return out
